# revision 99
# baseline (speedup 1.0000x reference)
"""Trainium2 Bass kernel for nn_Attention_81484119540519.

8-head attention block over 32x32 spatial (1024 tokens), C=512, B=16:
  qkv = BN(1x1conv(x)); S = q^T k * scale; P = softmax(S); A = v P^T
  pos = BN(depthwise3x3(v)); out = BN(1x1conv(A + pos))

Sharding: pure data-parallel over batch. B=16 -> 2 batches per core on 8
NeuronCores; no collectives. Host prepares permuted/folded weights, each
core computes its 2 batches, host concatenates.

Per-core dataflow (fp32 PSUM accumulation everywhere):
  - qk projection (bf16 matmuls) emits q/k in fp8e4, then SBUF->SBUF
    DMAs repack each head block to [16, 2, n] (channel pairs per
    partition, 32-aligned bases) for the PE's fp8 DoubleRow mode.
  - scores q^T k run as DoubleRow matmuls (K=16x2, 2x row rate,
    measured 2.07x on HW); exp on ScalarE straight out of PSUM (scale
    folded into the activation), writing fp8e4 E into [128, 2048]
    key-m-PAIR tiles.
  - v projection in natural channel order bf16 (the depthwise conv and
    pos branch dominate output magnitude, so v stays high precision);
    PE-transposed into fp8 v1T pair tiles [128, 2x640] (head slots
    padded 65->80 for the dual-fp8 Ldweights 16B-alignment rule) with a
    ones column per head: AV runs as DoubleRow matmuls contracting two
    key tiles per instruction, yielding A' = [A; Z] with Z the softmax
    denominator.  fp8 on the E/v1t/qk path costs ~0.4% rel err (the
    attended branch is a convex average, small next to pos).
  - 1/Z via reciprocal_approx_fast on a [2, HW] tile at base partition
    0 (the DVE ISA op corrupts at base 64 on HW), bounced through DRAM
    and broadcast across partitions with a stride-0 DMA read, then one
    tensor_mul per head merges A/Z; conv PSUM + BN bias fold in via
    scalar_tensor_tensor.
  - depthwise 3x3 conv: all 9 taps as diagonal-weight matmuls on
    TensorE over a y-padded image (PE has slack; DVE is the real-HW
    secondary bottleneck); only the x-edge wraparound corrections stay
    on DVE.
  - emission scheduling: the Act engine paces everything (128 exps),
    and the in-order PE queue executes in emission order, so all
    non-score work (pair tails, next batch's front, batch-0 outproj)
    is queued as ~1us chunks on a deque drained one per score slot --
    filling the PE/DVE idle windows under the exp stream instead of
    piling up at batch edges.  Per-head AV(3)+copies thread into the
    next head's emission; batch-0 pair 0 runs scores before the v path
    exists (AVs deferred); the last batch-0 tail is emitted inside
    batch-1's attention.
"""

import numpy as np
import ml_dtypes

NUM_HEADS = 8
KD = 32
HD = 64
C = 512
HW = 1024
SCALE = KD ** -0.5
B_PER_CORE = 2
N_CORES = 8

_cache = {}
CFG = dict(zb_dma=True, pool_dy0=False, az_pool=False, slots=True)


def _build_nc(loop_k=None, cfg=None):
    cfg = dict(CFG, **(cfg or {}))
    import concourse.bass as bass
    import concourse.tile as tile
    from concourse import bacc, mybir

    f32 = mybir.dt.float32
    bf16 = mybir.dt.bfloat16
    fp8 = mybir.dt.float8e4
    AF = mybir.ActivationFunctionType
    OP = mybir.AluOpType
    DR = mybir.MatmulPerfMode.DoubleRow

    nc = bacc.Bacc("TRN2", target_bir_lowering=False, debug=False)

    # ---- DRAM parameters (per-core shard + shared prepped weights) ----
    x_ext = nc.declare_dram_parameter("x", [B_PER_CORE, C, HW], bf16, isOutput=False)
    wqk_ext = nc.declare_dram_parameter("wqkT", [C, 512], bf16, isOutput=False)
    wv_ext = nc.declare_dram_parameter("wvT", [C, 512], bf16, isOutput=False)
    wo_ext = nc.declare_dram_parameter("woT", [C, 512], bf16, isOutput=False)
    # biases packed [128, 4] (column t = c-tile t)
    bqk_ext = nc.declare_dram_parameter("bqk", [128, 4], f32, isOutput=False)
    bv_ext = nc.declare_dram_parameter("bv", [128, 4], f32, isOutput=False)
    bo_ext = nc.declare_dram_parameter("bo", [128, 4], f32, isOutput=False)
    bpos_ext = nc.declare_dram_parameter("bpos", [128, 4], f32, isOutput=False)
    # diag conv weights [4 ctiles, 9 taps, 128, 128] bf16
    wdiag_ext = nc.declare_dram_parameter("wdiag", [4, 9, 128, 128], bf16, isOutput=False)
    # negated bf16-rounded conv weights for edge corrections [128, 4*9] f32
    wneg_ext = nc.declare_dram_parameter("wneg", [128, 36], f32, isOutput=False)
    wposc_ext = nc.declare_dram_parameter("wposc", [128, 36], f32, isOutput=False)
    ident_ext = nc.declare_dram_parameter("ident", [128, 128], bf16, isOutput=False)
    ones64_ext = nc.declare_dram_parameter("ones64", [65, 128], bf16, isOutput=False)
    out_ext = nc.declare_dram_parameter("out", [B_PER_CORE, C, HW], bf16, isOutput=True)
    zdram = nc.dram_tensor("zscratch", [16, HW], f32)
    if cfg.get("dbg"):
        dbga_ext = nc.declare_dram_parameter("dbga", [8, 65, HW], f32, isOutput=True)
        dbgrz_ext = nc.declare_dram_parameter("dbgrz", [8, HW], bf16, isOutput=True)
        dbge_ext = nc.declare_dram_parameter("dbge", [4, 128, HW], bf16, isOutput=True)

    NB = B_PER_CORE
    NM = 8           # key m tiles of 128
    NMP = 4          # key m-tile PAIRS
    NCHUNK = 2       # n chunks of 512
    VP = 1120        # padded v row length (zeros at [0,33) and [1057,1120))

    from contextlib import ExitStack

    with tile.TileContext(nc) as tc, ExitStack() as ctx:
        consts = ctx.enter_context(tc.tile_pool(name="consts", bufs=1))
        xbp = ctx.enter_context(tc.tile_pool(name="xb", bufs=8))
        qk8p = ctx.enter_context(tc.tile_pool(name="qk8", bufs=8))
        qk2p = ctx.enter_context(tc.tile_pool(name="qk2", bufs=8))
        vpp = ctx.enter_context(tc.tile_pool(name="vp", bufs=8))
        v1tp = ctx.enter_context(tc.tile_pool(name="v1t", bufs=8))
        ep = ctx.enter_context(tc.tile_pool(name="E", bufs=10))
        a65p = ctx.enter_context(tc.tile_pool(name="a65", bufs=5))
        rcpp = ctx.enter_context(tc.tile_pool(name="rcp", bufs=3))
        enhp = ctx.enter_context(tc.tile_pool(name="enh", bufs=8))
        outp = ctx.enter_context(tc.tile_pool(name="osb", bufs=4))

        s_psum = ctx.enter_context(tc.tile_pool(name="spsum", bufs=2, space="PSUM"))
        acc_psum = ctx.enter_context(tc.tile_pool(name="accpsum", bufs=2, space="PSUM"))
        misc_psum = ctx.enter_context(tc.tile_pool(name="miscpsum", bufs=2, space="PSUM"))

        if True:
            # ---------------- constants (DMA'd in order of first use) ----------
            wqk_sb = consts.tile([128, 4, 512], bf16)
            wv_sb = consts.tile([128, 4, 512], bf16)
            wo_sb = consts.tile([128, 4, 512], bf16)
            bqk_sb = consts.tile([128, 4], f32)
            bv_sb = consts.tile([128, 4], f32)
            bo_sb = consts.tile([128, 4], f32)
            bpos_sb = consts.tile([128, 4], f32)
            wdiag_sb = consts.tile([128, 4, 9, 128], bf16)
            wneg_sb = consts.tile([128, 36], f32)
            wposc_sb = consts.tile([128, 36], f32)
            ident_sb = consts.tile([128, 128], bf16)
            ones64_sb = consts.tile([65, 128], bf16)

            def emit_consts_early():
                nc.sync.dma_start(out=bqk_sb[:], in_=bqk_ext[:])
                for t in range(4):
                    nc.sync.dma_start(out=wqk_sb[:, t, :], in_=wqk_ext[t * 128:(t + 1) * 128, :])

            def emit_consts_v():
                nc.sync.dma_start(out=bv_sb[:], in_=bv_ext[:])
                for t in range(4):
                    nc.sync.dma_start(out=wv_sb[:, t, :], in_=wv_ext[t * 128:(t + 1) * 128, :])
                nc.sync.dma_start(out=ident_sb[:], in_=ident_ext[:])
                nc.sync.dma_start(out=ones64_sb[:], in_=ones64_ext[:])

            def emit_consts_late():
                nc.sync.dma_start(out=bpos_sb[:], in_=bpos_ext[:])
                nc.sync.dma_start(out=wneg_sb[:], in_=wneg_ext[:])
                nc.sync.dma_start(out=wposc_sb[:], in_=wposc_ext[:])
                for t in range(4):
                    nc.sync.dma_start(out=wdiag_sb[:, t, :, :], in_=wdiag_ext[t, :, :, :].rearrange("k p f -> p k f"))
                nc.sync.dma_start(out=bo_sb[:], in_=bo_ext[:])
                for t in range(4):
                    nc.sync.dma_start(out=wo_sb[:, t, :], in_=wo_ext[t * 128:(t + 1) * 128, :])

            def emit_front_xqk(b, ots=(0, 2, 1, 3), xb_t=None, defer=False):
                """x DMA, qk projection + fp8 repack for the given o-tiles.
                defer=True queues the work as defq chunks (next batch's
                front drains through the current batch's score slots)."""
                if xb_t is None:
                    xb_t = []
                    for kt in range(4):
                        xb = xbp.tile([128, HW], bf16)
                        # SWDGE: keeps x off the HWDGE queues that carry
                        # weights, so batch 0's projection isn't stuck
                        # behind const loads
                        nc.gpsimd.dma_start(out=xb[:], in_=x_ext[b, kt * 128:(kt + 1) * 128, :])
                        xb_t.append(xb)

                qk2_t = [None] * 4
                qk8_t = [None] * 4

                def repack(ot, blk):
                    # repack for DoubleRow: head block blk (32 channels) ->
                    # [16, 2, HW] (channel d=2p+i on partition p, slot i).
                    # AP base partitions only allow {0, 32, 64}, so blocks
                    # go in two [64, .] tiles at bases 0 and 32.
                    eng = nc.gpsimd if cfg.get("repack_swdge") else nc.sync
                    eng.dma_start(
                        out=qk2_t[ot][blk // 2][32 * (blk % 2): 32 * (blk % 2) + 16, :],
                        in_=qk8_t[ot][32 * blk: 32 * blk + 32, :])

                def proj_ch(ot, ch):
                    ps = misc_psum.tile([128, 512], f32, tag="mm")
                    for kt in range(4):
                        nc.tensor.matmul(
                            ps[:], wqk_sb[:, kt, ot * 128:(ot + 1) * 128],
                            xb_t[kt][:, ch * 512:(ch + 1) * 512],
                            start=(kt == 0), stop=(kt == 3))
                    nc.vector.tensor_scalar_add(
                        out=qk8_t[ot][:, ch * 512:(ch + 1) * 512], in0=ps[:],
                        scalar1=bqk_sb[:, ot:ot + 1])

                for ot in ots:  # head 0 needs tiles 0 (q) and 2 (k) first
                    qk8_t[ot] = qk8p.tile([128, HW], fp8, name="qk8")
                    qk2_t[ot] = [qk2p.tile([64, 2 * HW], fp8, name="qk2")
                                 for _ in range(2)]

                def unit(f):
                    qchunks([f]) if defer else f()

                for ot in ots:
                    unit(lambda ot=ot: proj_ch(ot, 0))
                    # blocks 0,1 (first head pair of each tile) repacked
                    # right after the tile; blocks 2,3 can trail
                    unit(lambda ot=ot: (proj_ch(ot, 1), repack(ot, 0),
                                        repack(ot, 1)))
                for ot in ots:
                    unit(lambda ot=ot: (repack(ot, 2), repack(ot, 3)))
                return xb_t, qk2_t

            def emit_front_v(b, xb_t, defer=False):
                """v projection (padded spatial layout) + v1T pair tiles."""
                if b == 0:
                    emit_consts_v()
                vp_t = [vpp.tile([128, VP], bf16, name="vp") for _ in range(4)]
                # v1T pair tiles: [128, 2x520-ish] fp8, m=2mp+i at free
                # 640*i, head h at h*80 (padded 65->80 so DoubleRow
                # Ldweights sees 16B-aligned offsets), ones column at +64.
                v1t_mp = [v1tp.tile([128, 1280], fp8, name="v1t")
                          for _ in range(NMP)]

                def unit(f):
                    qchunks([f]) if defer else f()

                def vproj_ch(ot, ch):
                    if ch == 0:
                        nc.gpsimd.memset(vp_t[ot][:, 0:33], 0.0)
                        nc.gpsimd.memset(vp_t[ot][:, 1057:1120], 0.0)
                    ps = misc_psum.tile([128, 512], f32, tag="mm")
                    for kt in range(4):
                        nc.tensor.matmul(
                            ps[:], wv_sb[:, kt, ot * 128:(ot + 1) * 128],
                            xb_t[kt][:, ch * 512:(ch + 1) * 512],
                            start=(kt == 0), stop=(kt == 3))
                    nc.vector.tensor_scalar_add(
                        out=vp_t[ot][:, 33 + ch * 512: 33 + (ch + 1) * 512], in0=ps[:],
                        scalar1=bv_sb[:, ot:ot + 1])

                def v1t_half(mp, par):
                    v1t = v1t_mp[mp]
                    if par == 0:
                        nc.gpsimd.memset(
                            v1t.rearrange("p (s c) -> p s c", s=16)[:, :, 64:65], 1.0)
                    m = 2 * mp + par
                    for ct in range(4):
                        tp = misc_psum.tile([128, 128], bf16, tag="mm")
                        nc.tensor.transpose(
                            tp[:], vp_t[ct][:, 33 + m * 128: 33 + (m + 1) * 128],
                            ident_sb[:])
                        nc.vector.tensor_copy(
                            out=v1t[:, 640 * par + 160 * ct: 640 * par + 160 * ct + 160]
                                .rearrange("p (s c) -> p s c", s=2)[:, :, 0:64],
                            in_=tp.rearrange("p (s c) -> p s c", s=2)[:, :, :])

                for ot in range(4):
                    for ch in range(NCHUNK):
                        unit(lambda ot=ot, ch=ch: vproj_ch(ot, ch))
                for mp in range(NMP):
                    for par in range(2):
                        unit(lambda mp=mp, par=par: v1t_half(mp, par))
                return vp_t, v1t_mp

            import collections
            defq = collections.deque()

            def qchunks(chunks):
                """Queue chunks for slot-draining, or emit inline when the
                slot mechanism is disabled (ablation)."""
                if cfg["slots"]:
                    defq.extend(chunks)
                else:
                    for c in chunks:
                        c()

            def fill_slot():
                """Emit one deferred chunk into the engines' idle window
                (the PE is Act-paced during scores, so deferred tail /
                outproj / next-front work queued here lands in gaps instead
                of piling up at the end of the batch).  Drains two chunks
                when backlogged."""
                if not cfg["slots"]:
                    return
                if defq:
                    defq.popleft()()
                if len(defq) > 8:
                    defq.popleft()()

            def flush_defq():
                while defq:
                    defq.popleft()()

            def emit_pair_tail(b, ct, pair_a65, pair_rz, vp_t):
                """normalize pair ct (PE base-64 ones broadcast of 1/Z from
                the a65 tile's own partition-64 row — no partition-moving
                DMA), then this c-tile's depthwise conv and merge.  Work is
                queued as small chunks on defq, drained one per score slot."""
                enh = enhp.tile([128, HW], bf16)

                def c_z(ch):
                    for hh in range(2):
                        zps = misc_psum.tile([128, 512], f32, tag="mm")
                        nc.tensor.matmul(
                            zps[:], ones64_sb[64:65, :],
                            pair_rz[hh][64:65, ch * 512:(ch + 1) * 512],
                            start=True, stop=True)
                        rcp = rcpp.tile([128, 512], f32, name="rcp")
                        nc.vector.reciprocal_approx_fast(out=rcp[:], in_=zps[:])
                        nc.vector.tensor_mul(
                            out=enh[hh * 64:(hh + 1) * 64, ch * 512:(ch + 1) * 512],
                            in0=pair_a65[hh][0:64, ch * 512:(ch + 1) * 512],
                            in1=rcp[0:64, :])

                def c_zdram_w():
                    # proven old scheme: Z rows DMA'd to partitions 0/1,
                    # recip [2, HW], bounce via DRAM
                    zpair = rcpp.tile([2, HW], f32, name="zpair", tag="zpair")
                    for hh in range(2):
                        nc.sync.dma_start(out=zpair[hh:hh + 1, :],
                                          in_=pair_a65[hh][64:65, :])
                    rzpair = rcpp.tile([2, HW], f32, name="rzpair", tag="rzpair")
                    nc.vector.reciprocal_approx_fast(out=rzpair[:], in_=zpair[:])
                    p0b = b * 8 + 2 * ct
                    nc.sync.dma_start(out=zdram[p0b:p0b + 2, :], in_=rzpair[:])

                def c_zdram(hh):
                    p0b = b * 8 + 2 * ct
                    zrow = zdram[p0b + hh:p0b + hh + 1, :]
                    bcast = bass.AP(tensor=zrow.tensor, offset=zrow.offset,
                                    ap=[[0, 64]] + list(zrow.ap[1:]))
                    zb = rcpp.tile([64, HW], f32, name="zb")
                    nc.sync.dma_start(out=zb[:], in_=bcast)
                    nc.vector.tensor_mul(
                        out=enh[hh * 64:(hh + 1) * 64, :],
                        in0=pair_a65[hh][0:64, :], in1=zb[:])

                dve_dy0 = cfg["pool_dy0"] and ct != 3
                pe_taps = [(ti, dy, dx) for ti, (dy, dx) in enumerate(
                    (dy, dx) for dy in (-1, 0, 1) for dx in (-1, 0, 1))
                    if not (dve_dy0 and dy == 0)]

                def c_conv(ch):
                    ps = misc_psum.tile([128, 512], f32, tag="mm")
                    for j, (ti, dy, dx) in enumerate(pe_taps):
                        off = 33 + 32 * dy + dx + ch * 512
                        nc.tensor.matmul(
                            ps[:], wdiag_sb[:, ct, ti, :],
                            vp_t[ct][:, off:off + 512],
                            start=(j == 0), stop=(j == len(pe_taps) - 1))
                    nc.vector.scalar_tensor_tensor(
                        out=enh[:, ch * 512:(ch + 1) * 512],
                        in0=ps[:], scalar=bpos_sb[:, ct:ct + 1],
                        in1=enh[:, ch * 512:(ch + 1) * 512],
                        op0=OP.add, op1=OP.add)

                def c_dy0():
                    # dy=0 conv row on DVE: enh += w * v (taps 3,4,5)
                    for dx in (-1, 0, 1):
                        ti = 3 + dx + 1
                        nc.vector.scalar_tensor_tensor(
                            out=enh[:], in0=vp_t[ct][:, 33 + dx: 33 + dx + HW],
                            scalar=wposc_sb[:, ct * 9 + ti: ct * 9 + ti + 1],
                            in1=enh[:], op0=OP.mult, op1=OP.add)

                def c_corr():
                    # x-wraparound corrections (dx = +/-1 taps)
                    for dy in (-1, 0, 1):
                        ys = [y for y in range(32) if 0 <= y + dy + 1 < 32]
                        y0, cnt = ys[0], len(ys)
                        oc = enh[:, y0 * 32: (y0 + cnt) * 32] \
                            .rearrange("p (a o) -> p a o", o=32)[:, :, 31:32]
                        sc = vp_t[ct][:, 33 + (y0 + dy + 1) * 32: 33 + (y0 + dy + 1 + cnt) * 32] \
                            .rearrange("p (a o) -> p a o", o=32)[:, :, 0:1]
                        nc.vector.scalar_tensor_tensor(
                            out=oc, in0=sc,
                            scalar=wneg_sb[:, ct * 9 + (dy + 1) * 3 + 2: ct * 9 + (dy + 1) * 3 + 3],
                            in1=oc, op0=OP.mult, op1=OP.add)
                        ys = [y for y in range(32) if 0 <= y + dy - 1 < 32]
                        y0, cnt = ys[0], len(ys)
                        oc = enh[:, y0 * 32: (y0 + cnt) * 32] \
                            .rearrange("p (a o) -> p a o", o=32)[:, :, 0:1]
                        sc = vp_t[ct][:, 33 + (y0 + dy - 1) * 32: 33 + (y0 + dy - 1 + cnt) * 32] \
                            .rearrange("p (a o) -> p a o", o=32)[:, :, 31:32]
                        nc.vector.scalar_tensor_tensor(
                            out=oc, in0=sc,
                            scalar=wneg_sb[:, ct * 9 + (dy + 1) * 3: ct * 9 + (dy + 1) * 3 + 1],
                            in1=oc, op0=OP.mult, op1=OP.add)

                if cfg["zb_dma"]:
                    chunks = [c_zdram_w, lambda: c_zdram(0), lambda: c_zdram(1),
                              lambda: c_conv(0), lambda: c_conv(1)]
                else:
                    chunks = [lambda: c_z(0), lambda: c_z(1),
                              lambda: c_conv(0), lambda: c_conv(1)]
                if dve_dy0:
                    chunks.append(c_dy0)
                chunks.append(c_corr)
                if cfg.get("dbg") and b == 0:
                    chunks.append(lambda: nc.sync.dma_start(
                        out=dbge_ext[ct, :, :], in_=enh[:]))
                qchunks(chunks)
                return enh

            def emit_head(h, qk2_t, fin_box, defer_av=False):
                """One head: DoubleRow scores per key-m tile, exp into fp8
                m-pair E tiles, DoubleRow AV contracting a pair per matmul.
                AV(mp) is emitted after scores(2mp+2) so the PE never waits
                on the exp of the pair it is about to consume; AV(3) plus
                the PSUM->SBUF copies move into a `finisher` that the NEXT
                head emits after its first exp, so the head-boundary
                Act->PE->Act latency hides under live work.  `fin_box`
                carries that pending finisher.  With defer_av the caller
                gets a closure emitting all AVs later (batch-0 startup:
                scores can run before the v path exists)."""
                t = h // 4
                pq = 32 * (h % 2)
                q2 = qk2_t[t][(h % 4) // 2].rearrange("p (s n) -> p s n", s=2)[pq:pq + 16, :, :]
                k2 = qk2_t[2 + t][(h % 4) // 2].rearrange("p (s n) -> p s n", s=2)[pq:pq + 16, :, :]
                a65c = []
                e2_mp = [None] * NMP

                def av(mp, v1t_mp):
                    if mp == 0:
                        a65c.extend(acc_psum.tile([65, 512], f32, name="a65c",
                                                  tag="a65c") for _ in range(NCHUNK))
                    w2 = v1t_mp[mp].rearrange("p (s c) -> p s c", s=2)[:, :, h * 80:h * 80 + 65]
                    e2v = e2_mp[mp].rearrange("p (s n) -> p s n", s=2)
                    for ch in range(NCHUNK):
                        nc.tensor.matmul(
                            a65c[ch][:], w2, e2v[:, :, ch * 512:(ch + 1) * 512],
                            start=(mp == 0), stop=(mp == NMP - 1), perf_mode=DR)

                def finish(v1t_mp):
                    av(NMP - 1, v1t_mp)
                    a65_sb = a65p.tile([65, HW], f32, name="a65_sb")
                    for ch in range(NCHUNK):
                        nc.vector.tensor_copy(
                            out=a65_sb[:, ch * 512:(ch + 1) * 512], in_=a65c[ch][:])
                    # Z row to bf16 (plain DVE copy handles base partition
                    # 64; the reciprocal ISA op does NOT on hardware, so
                    # 1/Z is taken after the base-0 broadcast instead)
                    zb = a65p.tile([65, HW], bf16, name="zb16", tag="zb16")
                    nc.vector.tensor_copy(out=zb[64:65, :], in_=a65_sb[64:65, :])
                    return a65_sb, zb

                for m in range(NM):
                    mp, par = m // 2, m % 2
                    st = s_psum.tile([128, HW], f32, name="st")
                    for ch in range(NCHUNK):
                        nc.tensor.matmul(
                            st[:, ch * 512:(ch + 1) * 512],
                            k2[:, :, m * 128:(m + 1) * 128],
                            q2[:, :, ch * 512:(ch + 1) * 512],
                            start=True, stop=True, perf_mode=DR)
                    if par == 0:
                        e2_mp[mp] = ep.tile([128, 2 * HW], fp8, name="e2")
                    nc.scalar.activation(
                        out=e2_mp[mp][:, par * HW:(par + 1) * HW], in_=st[:],
                        func=AF.Exp, scale=float(SCALE))
                    if m == 1 and fin_box[0] is not None:
                        f, fin_box[0] = fin_box[0], None
                        f()
                    if not defer_av and m in (3, 5, 7):
                        av((m - 3) // 2, fin_box[1])
                    if m >= 2:
                        fill_slot()

                if defer_av:
                    def deferred(v1t_mp):
                        for mp in range(NMP - 1):
                            av(mp, v1t_mp)
                        return finish(v1t_mp)
                    return deferred
                return finish

            def emit_attn(b, qk2_t, vp_t, v1t_mp, fin_box, cbs,
                          first_pair=None, pending=None, final=False):
                """Pairs of heads; each pair's tail chunks are queued half a
                pair later and drained by subsequent score slots.  cbs maps
                hp -> callback emitted after the pair.  first_pair: prebuilt
                (pair_a65, zpair) for batch-0's deferred pair 0.  pending
                carries the last tail's (dest, args) across batches."""
                enh_t = []
                fin_box[1] = v1t_mp
                start_hp = 0

                def mk_fin(finish, pair_a65, pair_rz, hh, v1t, h):
                    def f():
                        a65_sb, rz = finish(v1t)
                        pair_a65[hh] = a65_sb
                        pair_rz[hh] = rz
                        if cfg.get("dbg") and b == 0:
                            nc.sync.dma_start(out=dbga_ext[h, :, :], in_=a65_sb[:])
                            nc.sync.dma_start(out=dbgrz_ext[h:h + 1, :], in_=rz[64:65, :])
                    return f  # noqa: the rz here is the bf16 Z row tile

                if first_pair is not None:
                    pending = (enh_t, (b, 0, *first_pair, vp_t))
                    start_hp = 1
                for hp in range(start_hp, 4):
                    pair_a65 = [None, None]
                    pair_rz = [None, None]
                    for hh in range(2):
                        finish = emit_head(2 * hp + hh, qk2_t, fin_box)
                        fin_box[0] = mk_fin(finish, pair_a65, pair_rz, hh, v1t_mp,
                                            2 * hp + hh)
                        if hh == 0 and pending is not None:
                            # queue the previous pair's tail chunks half a
                            # pair late: far enough that their z-chain deps
                            # are ready when drain slots reach them, early
                            # enough that slots exist to drain them
                            dest, args = pending
                            dest.append(emit_pair_tail(*args))
                            pending = None
                    pending = (enh_t, (b, hp, pair_a65, pair_rz, vp_t))
                    if hp in cbs:
                        cbs[hp]()
                if final:
                    if fin_box[0] is not None:
                        f, fin_box[0] = fin_box[0], None
                        f()
                    dest, args = pending
                    dest.append(emit_pair_tail(*args))
                    pending = None
                return enh_t, pending

            def emit_outproj(b, enh_t, wide=False, ots=(0, 1, 2, 3), defer=False):
                # wide=True: attention is over, borrow the idle s_pool banks
                # for 2 full o-tiles in flight.  defer=True queues one chunk
                # per o-tile on defq instead of emitting inline.
                if defer:
                    for ot in ots:
                        qchunks([lambda ot=ot: emit_outproj(b, enh_t, ots=(ot,))])
                    return
                for ot in ots:
                    osb = outp.tile([128, HW], bf16)
                    if wide:
                        pw = s_psum.tile([128, HW], f32, tag="st", name="st")
                        for ch in range(NCHUNK):
                            for kt in range(4):
                                nc.tensor.matmul(
                                    pw[:, ch * 512:(ch + 1) * 512],
                                    wo_sb[:, kt, ot * 128:(ot + 1) * 128],
                                    enh_t[kt][:, ch * 512:(ch + 1) * 512],
                                    start=(kt == 0), stop=(kt == 3))
                        nc.vector.tensor_scalar_add(
                            out=osb[:], in0=pw[:], scalar1=bo_sb[:, ot:ot + 1])
                    else:
                        for ch in range(NCHUNK):
                            ps = misc_psum.tile([128, 512], f32, tag="mm")
                            for kt in range(4):
                                nc.tensor.matmul(
                                    ps[:], wo_sb[:, kt, ot * 128:(ot + 1) * 128],
                                    enh_t[kt][:, ch * 512:(ch + 1) * 512],
                                    start=(kt == 0), stop=(kt == 3))
                            nc.vector.tensor_scalar_add(
                                out=osb[:, ch * 512:(ch + 1) * 512], in0=ps[:],
                                scalar1=bo_sb[:, ot:ot + 1])
                    nc.sync.dma_start(out=out_ext[b, ot * 128:(ot + 1) * 128, :], in_=osb[:])

            def emit_all():
                # software pipelining: batch 0 starts attention before its v
                # path is built (pair-0 AVs deferred); batch 1's front is
                # emitted in two chunks mid-attention(0); outproj(0) in two
                # chunks mid-attention(1)
                fin_box = [None, None]
                emit_consts_early()
                xb0, qk2_0 = emit_front_xqk(0)
                d0 = emit_head(0, qk2_0, fin_box, defer_av=True)
                d1 = emit_head(1, qk2_0, fin_box, defer_av=True)
                vp0, v1t_0 = emit_front_v(0, xb0)
                emit_consts_late()
                fin_box[1] = v1t_0
                a65_00, rz_00 = d0(v1t_0)
                a65_01, rz_01 = d1(v1t_0)
                if cfg.get("dbg"):
                    nc.sync.dma_start(out=dbga_ext[0, :, :], in_=a65_00[:])
                    nc.sync.dma_start(out=dbgrz_ext[0:1, :], in_=rz_00[64:65, :])
                    nc.sync.dma_start(out=dbga_ext[1, :, :], in_=a65_01[:])
                    nc.sync.dma_start(out=dbgrz_ext[1:2, :], in_=rz_01[64:65, :])
                box = {}

                def cb_front1():
                    box["xqk"] = emit_front_xqk(1, defer=True)
                    box["v"] = emit_front_v(1, box["xqk"][0], defer=True)

                enh0, pend = emit_attn(
                    0, qk2_0, vp0, v1t_0, fin_box,
                    cbs={1: cb_front1},
                    first_pair=([a65_00, a65_01], [rz_00, rz_01]))
                qk2_1 = box["xqk"][1]
                vp1, v1t_1 = box["v"]
                enh1, _ = emit_attn(
                    1, qk2_1, vp1, v1t_1, fin_box,
                    cbs={1: lambda: emit_outproj(0, enh0, ots=(0, 1), defer=True),
                         2: lambda: emit_outproj(0, enh0, ots=(2, 3), defer=True)},
                    pending=pend, final=True)
                flush_defq()
                emit_outproj(1, enh1, wide=True)

            if loop_k is None:
                emit_all()
            else:
                with tc.For_i(0, loop_k, 1):
                    emit_all()

    nc.finalize()
    return nc


def _host_prep(w_qkv, g_qkv, b_qkv, w_pos, g_pos, b_pos, w_out, g_out, b_out):
    bf16 = ml_dtypes.bfloat16
    perm_q = np.empty(256, np.int64)
    perm_k = np.empty(256, np.int64)
    for t in range(2):
        for p in range(128):
            h = 4 * t + p // 32
            d = p % 32
            perm_q[t * 128 + p] = h * 128 + d
            perm_k[t * 128 + p] = h * 128 + 32 + d
    perm_qk = np.concatenate([perm_q, perm_k])
    perm_v = np.array([h * 128 + 64 + d for h in range(8) for d in range(64)])

    wg = (w_qkv * g_qkv[:, None]).astype(np.float32)
    wqkT = np.ascontiguousarray(wg[perm_qk].T).astype(bf16)
    wvT = np.ascontiguousarray(wg[perm_v].T).astype(bf16)
    woT = np.ascontiguousarray((w_out * g_out[:, None]).T).astype(bf16)

    def pack_bias(v):
        return np.ascontiguousarray(v.reshape(4, 128).T).astype(np.float32)

    wpos = (w_pos[:, 0] * g_pos[:, None, None]).astype(np.float32)  # [512, 3, 3]
    wdiag = np.zeros((4, 9, 128, 128), np.float32)
    idx = np.arange(128)
    for t in range(4):
        for ti, (dy, dx) in enumerate((dy, dx) for dy in (-1, 0, 1) for dx in (-1, 0, 1)):
            wdiag[t, ti, idx, idx] = wpos[t * 128:(t + 1) * 128, dy + 1, dx + 1]
    wdiag = wdiag.astype(bf16)
    # negated bf16-rounded weights for corrections: [128, 4*9]
    wneg = np.zeros((128, 36), np.float32)
    for t in range(4):
        for ti in range(9):
            dy, dx = ti // 3 - 1, ti % 3 - 1
            wneg[:, t * 9 + ti] = -wpos[t * 128:(t + 1) * 128, dy + 1, dx + 1] \
                .astype(bf16).astype(np.float32)

    return dict(
        wqkT=wqkT, wvT=wvT, woT=woT,
        bqk=pack_bias(b_qkv[perm_qk]), bv=pack_bias(b_qkv[perm_v]),
        bo=pack_bias(b_out), bpos=pack_bias(b_pos),
        wdiag=wdiag, wneg=wneg, wposc=-wneg,
        ident=np.eye(128, dtype=bf16),
        ones64=np.concatenate(
            [np.zeros((64, 128), np.float32), np.ones((1, 128), np.float32)]
        ).astype(bf16),
    )


def kernel(x, w_qkv, g_qkv, b_qkv, w_pos, g_pos, b_pos, w_out, g_out, b_out,
           _trace=False):
    from concourse.bass_utils import run_bass_kernel_spmd

    x = np.asarray(x, np.float32)
    B, Cin, H, W = x.shape
    assert (B, Cin, H, W) == (16, 512, 32, 32)

    if "nc" not in _cache:
        _cache["nc"] = _build_nc()
    nc = _cache["nc"]

    prep = _host_prep(np.asarray(w_qkv, np.float32), np.asarray(g_qkv, np.float32),
                      np.asarray(b_qkv, np.float32), np.asarray(w_pos, np.float32),
                      np.asarray(g_pos, np.float32), np.asarray(b_pos, np.float32),
                      np.asarray(w_out, np.float32), np.asarray(g_out, np.float32),
                      np.asarray(b_out, np.float32))

    xs = x.reshape(N_CORES, B_PER_CORE, 512, 1024).astype(ml_dtypes.bfloat16)
    in_maps = [dict(prep, x=np.ascontiguousarray(xs[i])) for i in range(N_CORES)]
    _cache["last_in_maps"] = in_maps
    res = run_bass_kernel_spmd(nc, in_maps, list(range(N_CORES)))
    _cache["last_result"] = res
    out = np.stack([res.results[i]["out"] for i in range(N_CORES)])
    return out.reshape(16, 512, 32, 32).astype(np.float32)


# revision 104
# speedup vs baseline: 1.0243x; 1.0243x over previous
"""Trainium2 Bass kernel for nn_Attention_81484119540519.

8-head attention block over 32x32 spatial (1024 tokens), C=512, B=16:
  qkv = BN(1x1conv(x)); S = q^T k * scale; P = softmax(S); A = v P^T
  pos = BN(depthwise3x3(v)); out = BN(1x1conv(A + pos))

Sharding: pure data-parallel over batch. B=16 -> 2 batches per core on 8
NeuronCores; no collectives. Host prepares permuted/folded weights, each
core computes its 2 batches, host concatenates.

Per-core dataflow (fp32 PSUM accumulation everywhere):
  - qk projection (bf16 matmuls) emits q/k in fp8e4, then SBUF->SBUF
    DMAs repack each head block to [16, 2, n] (channel pairs per
    partition, 32-aligned bases) for the PE's fp8 DoubleRow mode.
  - scores q^T k run as DoubleRow matmuls (K=16x2, 2x row rate,
    measured 2.07x on HW); exp on ScalarE straight out of PSUM (scale
    folded into the activation), writing fp8e4 E into [128, 2048]
    key-m-PAIR tiles.
  - v projection in natural channel order bf16 (the depthwise conv and
    pos branch dominate output magnitude, so v stays high precision);
    PE-transposed into fp8 v1T pair tiles [128, 2x640] (head slots
    padded 65->80 for the dual-fp8 Ldweights 16B-alignment rule) with a
    ones column per head: AV runs as DoubleRow matmuls contracting two
    key tiles per instruction, yielding A' = [A; Z] with Z the softmax
    denominator.  fp8 on the E/v1t/qk path costs ~0.4% rel err (the
    attended branch is a convex average, small next to pos).
  - 1/Z via reciprocal_approx_fast on a [2, HW] tile at base partition
    0 (the DVE ISA op corrupts at base 64 on HW), bounced through DRAM
    and broadcast across partitions with a stride-0 DMA read, then one
    tensor_mul per head merges A/Z; conv PSUM + BN bias fold in via
    scalar_tensor_tensor.
  - depthwise 3x3 conv: all 9 taps as diagonal-weight matmuls on
    TensorE over a y-padded image (PE has slack; DVE is the real-HW
    secondary bottleneck); only the x-edge wraparound corrections stay
    on DVE.
  - emission scheduling: the Act engine paces everything (128 exps),
    and the in-order PE queue executes in emission order, so all
    non-score work (pair tails, next batch's front, batch-0 outproj)
    is queued as ~1us chunks on a deque drained one per score slot --
    filling the PE/DVE idle windows under the exp stream instead of
    piling up at batch edges.  Per-head AV(3)+copies thread into the
    next head's emission; batch-0 pair 0 runs scores before the v path
    exists (AVs deferred); the last batch-0 tail is emitted inside
    batch-1's attention.
"""

import numpy as np
import ml_dtypes

NUM_HEADS = 8
KD = 32
HD = 64
C = 512
HW = 1024
SCALE = KD ** -0.5
B_PER_CORE = 2
N_CORES = 8

_cache = {}
CFG = dict(zb_dma=True, pool_dy0=False, az_pool=False, slots=True, z_swdge=True)


def _build_nc(loop_k=None, cfg=None):
    cfg = dict(CFG, **(cfg or {}))
    import concourse.bass as bass
    import concourse.tile as tile
    from concourse import bacc, mybir

    f32 = mybir.dt.float32
    bf16 = mybir.dt.bfloat16
    fp8 = mybir.dt.float8e4
    AF = mybir.ActivationFunctionType
    OP = mybir.AluOpType
    DR = mybir.MatmulPerfMode.DoubleRow

    nc = bacc.Bacc("TRN2", target_bir_lowering=False, debug=False)

    # ---- DRAM parameters (per-core shard + shared prepped weights) ----
    x_ext = nc.declare_dram_parameter("x", [B_PER_CORE, C, HW], bf16, isOutput=False)
    wqk_ext = nc.declare_dram_parameter("wqkT", [C, 512], bf16, isOutput=False)
    wv_ext = nc.declare_dram_parameter("wvT", [C, 512], bf16, isOutput=False)
    wo_ext = nc.declare_dram_parameter("woT", [C, 512], bf16, isOutput=False)
    # biases packed [128, 4] (column t = c-tile t)
    bqk_ext = nc.declare_dram_parameter("bqk", [128, 4], f32, isOutput=False)
    bv_ext = nc.declare_dram_parameter("bv", [128, 4], f32, isOutput=False)
    bo_ext = nc.declare_dram_parameter("bo", [128, 4], f32, isOutput=False)
    bpos_ext = nc.declare_dram_parameter("bpos", [128, 4], f32, isOutput=False)
    # diag conv weights [4 ctiles, 9 taps, 128, 128] bf16
    wdiag_ext = nc.declare_dram_parameter("wdiag", [4, 9, 128, 128], bf16, isOutput=False)
    # negated bf16-rounded conv weights for edge corrections [128, 4*9] f32
    wneg_ext = nc.declare_dram_parameter("wneg", [128, 36], f32, isOutput=False)
    wposc_ext = nc.declare_dram_parameter("wposc", [128, 36], f32, isOutput=False)
    ident_ext = nc.declare_dram_parameter("ident", [128, 128], bf16, isOutput=False)
    ones64_ext = nc.declare_dram_parameter("ones64", [65, 128], bf16, isOutput=False)
    out_ext = nc.declare_dram_parameter("out", [B_PER_CORE, C, HW], bf16, isOutput=True)
    zdram = nc.dram_tensor("zscratch", [16, HW], f32)
    if cfg.get("dbg"):
        dbga_ext = nc.declare_dram_parameter("dbga", [8, 65, HW], f32, isOutput=True)
        dbgrz_ext = nc.declare_dram_parameter("dbgrz", [8, HW], bf16, isOutput=True)
        dbge_ext = nc.declare_dram_parameter("dbge", [4, 128, HW], bf16, isOutput=True)

    NB = B_PER_CORE
    NM = 8           # key m tiles of 128
    NMP = 4          # key m-tile PAIRS
    NCHUNK = 2       # n chunks of 512
    VP = 1120        # padded v row length (zeros at [0,33) and [1057,1120))

    from contextlib import ExitStack

    with tile.TileContext(nc) as tc, ExitStack() as ctx:
        consts = ctx.enter_context(tc.tile_pool(name="consts", bufs=1))
        xbp = ctx.enter_context(tc.tile_pool(name="xb", bufs=8))
        qk8p = ctx.enter_context(tc.tile_pool(name="qk8", bufs=8))
        qk2p = ctx.enter_context(tc.tile_pool(name="qk2", bufs=8))
        vpp = ctx.enter_context(tc.tile_pool(name="vp", bufs=8))
        v1tp = ctx.enter_context(tc.tile_pool(name="v1t", bufs=8))
        ep = ctx.enter_context(tc.tile_pool(name="E", bufs=10))
        a65p = ctx.enter_context(tc.tile_pool(name="a65", bufs=5))
        rcpp = ctx.enter_context(tc.tile_pool(name="rcp", bufs=3))
        enhp = ctx.enter_context(tc.tile_pool(name="enh", bufs=8))
        outp = ctx.enter_context(tc.tile_pool(name="osb", bufs=4))

        s_psum = ctx.enter_context(tc.tile_pool(name="spsum", bufs=2, space="PSUM"))
        acc_psum = ctx.enter_context(tc.tile_pool(name="accpsum", bufs=2, space="PSUM"))
        misc_psum = ctx.enter_context(tc.tile_pool(name="miscpsum", bufs=2, space="PSUM"))

        if True:
            # ---------------- constants (DMA'd in order of first use) ----------
            wqk_sb = consts.tile([128, 4, 512], bf16)
            wv_sb = consts.tile([128, 4, 512], bf16)
            wo_sb = consts.tile([128, 4, 512], bf16)
            bqk_sb = consts.tile([128, 4], f32)
            bv_sb = consts.tile([128, 4], f32)
            bo_sb = consts.tile([128, 4], f32)
            bpos_sb = consts.tile([128, 4], f32)
            wdiag_sb = consts.tile([128, 4, 9, 128], bf16)
            wneg_sb = consts.tile([128, 36], f32)
            wposc_sb = consts.tile([128, 36], f32)
            ident_sb = consts.tile([128, 128], bf16)
            ones64_sb = consts.tile([65, 128], bf16)

            def emit_consts_early():
                nc.sync.dma_start(out=bqk_sb[:], in_=bqk_ext[:])
                for t in range(4):
                    nc.sync.dma_start(out=wqk_sb[:, t, :], in_=wqk_ext[t * 128:(t + 1) * 128, :])

            def emit_consts_v():
                nc.sync.dma_start(out=bv_sb[:], in_=bv_ext[:])
                for t in range(4):
                    nc.sync.dma_start(out=wv_sb[:, t, :], in_=wv_ext[t * 128:(t + 1) * 128, :])
                nc.sync.dma_start(out=ident_sb[:], in_=ident_ext[:])
                nc.sync.dma_start(out=ones64_sb[:], in_=ones64_ext[:])

            def emit_consts_late():
                nc.sync.dma_start(out=bpos_sb[:], in_=bpos_ext[:])
                nc.sync.dma_start(out=wneg_sb[:], in_=wneg_ext[:])
                nc.sync.dma_start(out=wposc_sb[:], in_=wposc_ext[:])
                for t in range(4):
                    nc.sync.dma_start(out=wdiag_sb[:, t, :, :], in_=wdiag_ext[t, :, :, :].rearrange("k p f -> p k f"))
                nc.sync.dma_start(out=bo_sb[:], in_=bo_ext[:])
                for t in range(4):
                    nc.sync.dma_start(out=wo_sb[:, t, :], in_=wo_ext[t * 128:(t + 1) * 128, :])

            def emit_front_xqk(b, ots=(0, 2, 1, 3), xb_t=None, defer=False):
                """x DMA, qk projection + fp8 repack for the given o-tiles.
                defer=True queues the work as defq chunks (next batch's
                front drains through the current batch's score slots)."""
                if xb_t is None:
                    xb_t = []
                    for kt in range(4):
                        xb = xbp.tile([128, HW], bf16)
                        # SWDGE: keeps x off the HWDGE queues that carry
                        # weights, so batch 0's projection isn't stuck
                        # behind const loads
                        nc.gpsimd.dma_start(out=xb[:], in_=x_ext[b, kt * 128:(kt + 1) * 128, :])
                        xb_t.append(xb)

                qk2_t = [None] * 4
                qk8_t = [None] * 4

                def repack(ot, blk):
                    # repack for DoubleRow: head block blk (32 channels) ->
                    # [16, 2, HW] (channel d=2p+i on partition p, slot i).
                    # AP base partitions only allow {0, 32, 64}, so blocks
                    # go in two [64, .] tiles at bases 0 and 32.
                    eng = nc.gpsimd if cfg.get("repack_swdge") else nc.sync
                    eng.dma_start(
                        out=qk2_t[ot][blk // 2][32 * (blk % 2): 32 * (blk % 2) + 16, :],
                        in_=qk8_t[ot][32 * blk: 32 * blk + 32, :])

                def proj_ch(ot, ch):
                    ps = misc_psum.tile([128, 512], f32, tag="mm")
                    for kt in range(4):
                        nc.tensor.matmul(
                            ps[:], wqk_sb[:, kt, ot * 128:(ot + 1) * 128],
                            xb_t[kt][:, ch * 512:(ch + 1) * 512],
                            start=(kt == 0), stop=(kt == 3))
                    nc.vector.tensor_scalar_add(
                        out=qk8_t[ot][:, ch * 512:(ch + 1) * 512], in0=ps[:],
                        scalar1=bqk_sb[:, ot:ot + 1])

                for ot in ots:  # head 0 needs tiles 0 (q) and 2 (k) first
                    qk8_t[ot] = qk8p.tile([128, HW], fp8, name="qk8")
                    qk2_t[ot] = [qk2p.tile([64, 2 * HW], fp8, name="qk2")
                                 for _ in range(2)]

                def unit(f):
                    qchunks([f]) if defer else f()

                for ot in ots:
                    unit(lambda ot=ot: proj_ch(ot, 0))
                    # blocks 0,1 (first head pair of each tile) repacked
                    # right after the tile; blocks 2,3 can trail
                    unit(lambda ot=ot: (proj_ch(ot, 1), repack(ot, 0),
                                        repack(ot, 1)))
                for ot in ots:
                    unit(lambda ot=ot: (repack(ot, 2), repack(ot, 3)))
                return xb_t, qk2_t

            def emit_front_v(b, xb_t, defer=False):
                """v projection (padded spatial layout) + v1T pair tiles."""
                if b == 0:
                    emit_consts_v()
                vp_t = [vpp.tile([128, VP], bf16, name="vp") for _ in range(4)]
                # v1T pair tiles: [128, 2x520-ish] fp8, m=2mp+i at free
                # 640*i, head h at h*80 (padded 65->80 so DoubleRow
                # Ldweights sees 16B-aligned offsets), ones column at +64.
                v1t_mp = [v1tp.tile([128, 1280], fp8, name="v1t")
                          for _ in range(NMP)]

                def unit(f):
                    qchunks([f]) if defer else f()

                def vproj_ch(ot, ch):
                    if ch == 0:
                        nc.gpsimd.memset(vp_t[ot][:, 0:33], 0.0)
                        nc.gpsimd.memset(vp_t[ot][:, 1057:1120], 0.0)
                    ps = misc_psum.tile([128, 512], f32, tag="mm")
                    for kt in range(4):
                        nc.tensor.matmul(
                            ps[:], wv_sb[:, kt, ot * 128:(ot + 1) * 128],
                            xb_t[kt][:, ch * 512:(ch + 1) * 512],
                            start=(kt == 0), stop=(kt == 3))
                    nc.vector.tensor_scalar_add(
                        out=vp_t[ot][:, 33 + ch * 512: 33 + (ch + 1) * 512], in0=ps[:],
                        scalar1=bv_sb[:, ot:ot + 1])

                def v1t_half(mp, par):
                    v1t = v1t_mp[mp]
                    if par == 0:
                        nc.gpsimd.memset(
                            v1t.rearrange("p (s c) -> p s c", s=16)[:, :, 64:65], 1.0)
                    m = 2 * mp + par
                    for ct in range(4):
                        tp = misc_psum.tile([128, 128], bf16, tag="mm")
                        nc.tensor.transpose(
                            tp[:], vp_t[ct][:, 33 + m * 128: 33 + (m + 1) * 128],
                            ident_sb[:])
                        nc.vector.tensor_copy(
                            out=v1t[:, 640 * par + 160 * ct: 640 * par + 160 * ct + 160]
                                .rearrange("p (s c) -> p s c", s=2)[:, :, 0:64],
                            in_=tp.rearrange("p (s c) -> p s c", s=2)[:, :, :])

                for ot in range(4):
                    for ch in range(NCHUNK):
                        unit(lambda ot=ot, ch=ch: vproj_ch(ot, ch))
                for mp in range(NMP):
                    for par in range(2):
                        unit(lambda mp=mp, par=par: v1t_half(mp, par))
                return vp_t, v1t_mp

            import collections
            defq = collections.deque()

            def qchunks(chunks):
                """Queue chunks for slot-draining, or emit inline when the
                slot mechanism is disabled (ablation)."""
                if cfg["slots"]:
                    defq.extend(chunks)
                else:
                    for c in chunks:
                        c()

            def fill_slot():
                """Emit one deferred chunk into the engines' idle window
                (the PE is Act-paced during scores, so deferred tail /
                outproj / next-front work queued here lands in gaps instead
                of piling up at the end of the batch).  Drains two chunks
                when backlogged."""
                if not cfg["slots"]:
                    return
                if defq:
                    defq.popleft()()
                if len(defq) > 8:
                    defq.popleft()()

            def flush_defq():
                while defq:
                    defq.popleft()()

            def emit_pair_tail(b, ct, pair_a65, pair_rz, vp_t):
                """normalize pair ct (PE base-64 ones broadcast of 1/Z from
                the a65 tile's own partition-64 row — no partition-moving
                DMA), then this c-tile's depthwise conv and merge.  Work is
                queued as small chunks on defq, drained one per score slot."""
                enh = enhp.tile([128, HW], bf16)

                def c_z(ch):
                    for hh in range(2):
                        zps = misc_psum.tile([128, 512], f32, tag="mm")
                        nc.tensor.matmul(
                            zps[:], ones64_sb[64:65, :],
                            pair_rz[hh][64:65, ch * 512:(ch + 1) * 512],
                            start=True, stop=True)
                        rcp = rcpp.tile([128, 512], f32, name="rcp")
                        nc.vector.reciprocal_approx_fast(out=rcp[:], in_=zps[:])
                        nc.vector.tensor_mul(
                            out=enh[hh * 64:(hh + 1) * 64, ch * 512:(ch + 1) * 512],
                            in0=pair_a65[hh][0:64, ch * 512:(ch + 1) * 512],
                            in1=rcp[0:64, :])

                def c_zdram_w():
                    # proven old scheme: Z rows DMA'd to partitions 0/1,
                    # recip [2, HW], bounce via DRAM
                    zpair = rcpp.tile([2, HW], f32, name="zpair", tag="zpair")
                    for hh in range(2):
                        nc.sync.dma_start(out=zpair[hh:hh + 1, :],
                                          in_=pair_a65[hh][64:65, :])
                    rzpair = rcpp.tile([2, HW], f32, name="rzpair", tag="rzpair")
                    nc.vector.reciprocal_approx_fast(out=rzpair[:], in_=zpair[:])
                    p0b = b * 8 + 2 * ct
                    nc.sync.dma_start(out=zdram[p0b:p0b + 2, :], in_=rzpair[:])

                def c_zdram(hh):
                    p0b = b * 8 + 2 * ct
                    zrow = zdram[p0b + hh:p0b + hh + 1, :]
                    bcast = bass.AP(tensor=zrow.tensor, offset=zrow.offset,
                                    ap=[[0, 64]] + list(zrow.ap[1:]))
                    zb = rcpp.tile([64, HW], f32, name="zb")
                    # big broadcast reads optionally on the SWDGE queue to
                    # relieve the sync HWDGE queue (outputs/repacks/consts)
                    eng = nc.gpsimd if cfg.get("z_swdge") else nc.sync
                    eng.dma_start(out=zb[:], in_=bcast)
                    nc.vector.tensor_mul(
                        out=enh[hh * 64:(hh + 1) * 64, :],
                        in0=pair_a65[hh][0:64, :], in1=zb[:])

                dve_dy0 = cfg["pool_dy0"] and ct != 3
                pe_taps = [(ti, dy, dx) for ti, (dy, dx) in enumerate(
                    (dy, dx) for dy in (-1, 0, 1) for dx in (-1, 0, 1))
                    if not (dve_dy0 and dy == 0)]

                def c_conv(ch):
                    ps = misc_psum.tile([128, 512], f32, tag="mm")
                    for j, (ti, dy, dx) in enumerate(pe_taps):
                        off = 33 + 32 * dy + dx + ch * 512
                        nc.tensor.matmul(
                            ps[:], wdiag_sb[:, ct, ti, :],
                            vp_t[ct][:, off:off + 512],
                            start=(j == 0), stop=(j == len(pe_taps) - 1))
                    nc.vector.scalar_tensor_tensor(
                        out=enh[:, ch * 512:(ch + 1) * 512],
                        in0=ps[:], scalar=bpos_sb[:, ct:ct + 1],
                        in1=enh[:, ch * 512:(ch + 1) * 512],
                        op0=OP.add, op1=OP.add)

                def c_dy0():
                    # dy=0 conv row on DVE: enh += w * v (taps 3,4,5)
                    for dx in (-1, 0, 1):
                        ti = 3 + dx + 1
                        nc.vector.scalar_tensor_tensor(
                            out=enh[:], in0=vp_t[ct][:, 33 + dx: 33 + dx + HW],
                            scalar=wposc_sb[:, ct * 9 + ti: ct * 9 + ti + 1],
                            in1=enh[:], op0=OP.mult, op1=OP.add)

                def c_corr():
                    # x-wraparound corrections (dx = +/-1 taps)
                    for dy in (-1, 0, 1):
                        ys = [y for y in range(32) if 0 <= y + dy + 1 < 32]
                        y0, cnt = ys[0], len(ys)
                        oc = enh[:, y0 * 32: (y0 + cnt) * 32] \
                            .rearrange("p (a o) -> p a o", o=32)[:, :, 31:32]
                        sc = vp_t[ct][:, 33 + (y0 + dy + 1) * 32: 33 + (y0 + dy + 1 + cnt) * 32] \
                            .rearrange("p (a o) -> p a o", o=32)[:, :, 0:1]
                        nc.vector.scalar_tensor_tensor(
                            out=oc, in0=sc,
                            scalar=wneg_sb[:, ct * 9 + (dy + 1) * 3 + 2: ct * 9 + (dy + 1) * 3 + 3],
                            in1=oc, op0=OP.mult, op1=OP.add)
                        ys = [y for y in range(32) if 0 <= y + dy - 1 < 32]
                        y0, cnt = ys[0], len(ys)
                        oc = enh[:, y0 * 32: (y0 + cnt) * 32] \
                            .rearrange("p (a o) -> p a o", o=32)[:, :, 0:1]
                        sc = vp_t[ct][:, 33 + (y0 + dy - 1) * 32: 33 + (y0 + dy - 1 + cnt) * 32] \
                            .rearrange("p (a o) -> p a o", o=32)[:, :, 31:32]
                        nc.vector.scalar_tensor_tensor(
                            out=oc, in0=sc,
                            scalar=wneg_sb[:, ct * 9 + (dy + 1) * 3: ct * 9 + (dy + 1) * 3 + 1],
                            in1=oc, op0=OP.mult, op1=OP.add)

                if cfg["zb_dma"]:
                    chunks = [c_zdram_w, lambda: c_zdram(0), lambda: c_zdram(1),
                              lambda: c_conv(0), lambda: c_conv(1)]
                else:
                    chunks = [lambda: c_z(0), lambda: c_z(1),
                              lambda: c_conv(0), lambda: c_conv(1)]
                if dve_dy0:
                    chunks.append(c_dy0)
                chunks.append(c_corr)
                if cfg.get("dbg") and b == 0:
                    chunks.append(lambda: nc.sync.dma_start(
                        out=dbge_ext[ct, :, :], in_=enh[:]))
                qchunks(chunks)
                return enh

            def emit_head(h, qk2_t, fin_box, defer_av=False):
                """One head: DoubleRow scores per key-m tile, exp into fp8
                m-pair E tiles, DoubleRow AV contracting a pair per matmul.
                AV(mp) is emitted after scores(2mp+2) so the PE never waits
                on the exp of the pair it is about to consume; AV(3) plus
                the PSUM->SBUF copies move into a `finisher` that the NEXT
                head emits after its first exp, so the head-boundary
                Act->PE->Act latency hides under live work.  `fin_box`
                carries that pending finisher.  With defer_av the caller
                gets a closure emitting all AVs later (batch-0 startup:
                scores can run before the v path exists)."""
                t = h // 4
                pq = 32 * (h % 2)
                q2 = qk2_t[t][(h % 4) // 2].rearrange("p (s n) -> p s n", s=2)[pq:pq + 16, :, :]
                k2 = qk2_t[2 + t][(h % 4) // 2].rearrange("p (s n) -> p s n", s=2)[pq:pq + 16, :, :]
                a65c = []
                e2_mp = [None] * NMP

                def av(mp, v1t_mp):
                    if mp == 0:
                        a65c.extend(acc_psum.tile([65, 512], f32, name="a65c",
                                                  tag="a65c") for _ in range(NCHUNK))
                    w2 = v1t_mp[mp].rearrange("p (s c) -> p s c", s=2)[:, :, h * 80:h * 80 + 65]
                    e2v = e2_mp[mp].rearrange("p (s n) -> p s n", s=2)
                    for ch in range(NCHUNK):
                        nc.tensor.matmul(
                            a65c[ch][:], w2, e2v[:, :, ch * 512:(ch + 1) * 512],
                            start=(mp == 0), stop=(mp == NMP - 1), perf_mode=DR)

                def finish(v1t_mp):
                    av(NMP - 1, v1t_mp)
                    a65_sb = a65p.tile([65, HW], f32, name="a65_sb")
                    for ch in range(NCHUNK):
                        nc.vector.tensor_copy(
                            out=a65_sb[:, ch * 512:(ch + 1) * 512], in_=a65c[ch][:])
                    if cfg["zb_dma"]:
                        # Z reaches the tail via the a65 row-64 DMA; no
                        # bf16 Z-row copy needed
                        return a65_sb, None
                    # Z row to bf16 (plain DVE copy handles base partition
                    # 64; the reciprocal ISA op does NOT on hardware, so
                    # 1/Z is taken after the base-0 broadcast instead)
                    zb = a65p.tile([65, HW], bf16, name="zb16", tag="zb16")
                    nc.vector.tensor_copy(out=zb[64:65, :], in_=a65_sb[64:65, :])
                    return a65_sb, zb

                for m in range(NM):
                    mp, par = m // 2, m % 2
                    st = s_psum.tile([128, HW], f32, name="st")
                    for ch in range(NCHUNK):
                        nc.tensor.matmul(
                            st[:, ch * 512:(ch + 1) * 512],
                            k2[:, :, m * 128:(m + 1) * 128],
                            q2[:, :, ch * 512:(ch + 1) * 512],
                            start=True, stop=True, perf_mode=DR)
                    if par == 0:
                        e2_mp[mp] = ep.tile([128, 2 * HW], fp8, name="e2")
                    nc.scalar.activation(
                        out=e2_mp[mp][:, par * HW:(par + 1) * HW], in_=st[:],
                        func=AF.Exp, scale=float(SCALE))
                    if m == 1 and fin_box[0] is not None:
                        f, fin_box[0] = fin_box[0], None
                        f()
                    if not defer_av and m in (3, 5, 7):
                        av((m - 3) // 2, fin_box[1])
                    if m >= 2:
                        fill_slot()

                if defer_av:
                    def deferred(v1t_mp):
                        for mp in range(NMP - 1):
                            av(mp, v1t_mp)
                        return finish(v1t_mp)
                    return deferred
                return finish

            def emit_attn(b, qk2_t, vp_t, v1t_mp, fin_box, cbs,
                          first_pair=None, pending=None, final=False):
                """Pairs of heads; each pair's tail chunks are queued half a
                pair later and drained by subsequent score slots.  cbs maps
                hp -> callback emitted after the pair.  first_pair: prebuilt
                (pair_a65, zpair) for batch-0's deferred pair 0.  pending
                carries the last tail's (dest, args) across batches."""
                enh_t = []
                fin_box[1] = v1t_mp
                start_hp = 0

                def mk_fin(finish, pair_a65, pair_rz, hh, v1t, h):
                    def f():
                        a65_sb, rz = finish(v1t)
                        pair_a65[hh] = a65_sb
                        pair_rz[hh] = rz
                        if cfg.get("dbg") and b == 0:
                            nc.sync.dma_start(out=dbga_ext[h, :, :], in_=a65_sb[:])
                            if rz is not None:
                                nc.sync.dma_start(out=dbgrz_ext[h:h + 1, :],
                                                  in_=rz[64:65, :])
                    return f  # noqa: the rz here is the bf16 Z row tile

                if first_pair is not None:
                    pending = (enh_t, (b, 0, *first_pair, vp_t))
                    start_hp = 1
                for hp in range(start_hp, 4):
                    pair_a65 = [None, None]
                    pair_rz = [None, None]
                    for hh in range(2):
                        finish = emit_head(2 * hp + hh, qk2_t, fin_box)
                        fin_box[0] = mk_fin(finish, pair_a65, pair_rz, hh, v1t_mp,
                                            2 * hp + hh)
                        if hh == 0 and pending is not None:
                            # queue the previous pair's tail chunks half a
                            # pair late: far enough that their z-chain deps
                            # are ready when drain slots reach them, early
                            # enough that slots exist to drain them
                            dest, args = pending
                            dest.append(emit_pair_tail(*args))
                            pending = None
                    pending = (enh_t, (b, hp, pair_a65, pair_rz, vp_t))
                    if hp in cbs:
                        cbs[hp]()
                if final:
                    if fin_box[0] is not None:
                        f, fin_box[0] = fin_box[0], None
                        f()
                    dest, args = pending
                    dest.append(emit_pair_tail(*args))
                    pending = None
                return enh_t, pending

            def emit_outproj(b, enh_t, wide=False, ots=(0, 1, 2, 3), defer=False):
                # wide=True: attention is over, borrow the idle s_pool banks
                # for 2 full o-tiles in flight.  defer=True queues one chunk
                # per o-tile on defq instead of emitting inline.
                if defer:
                    for ot in ots:
                        qchunks([lambda ot=ot: emit_outproj(b, enh_t, ots=(ot,))])
                    return
                for ot in ots:
                    osb = outp.tile([128, HW], bf16)
                    if wide:
                        pw = s_psum.tile([128, HW], f32, tag="st", name="st")
                        for ch in range(NCHUNK):
                            for kt in range(4):
                                nc.tensor.matmul(
                                    pw[:, ch * 512:(ch + 1) * 512],
                                    wo_sb[:, kt, ot * 128:(ot + 1) * 128],
                                    enh_t[kt][:, ch * 512:(ch + 1) * 512],
                                    start=(kt == 0), stop=(kt == 3))
                        nc.vector.tensor_scalar_add(
                            out=osb[:], in0=pw[:], scalar1=bo_sb[:, ot:ot + 1])
                    else:
                        for ch in range(NCHUNK):
                            ps = misc_psum.tile([128, 512], f32, tag="mm")
                            for kt in range(4):
                                nc.tensor.matmul(
                                    ps[:], wo_sb[:, kt, ot * 128:(ot + 1) * 128],
                                    enh_t[kt][:, ch * 512:(ch + 1) * 512],
                                    start=(kt == 0), stop=(kt == 3))
                            nc.vector.tensor_scalar_add(
                                out=osb[:, ch * 512:(ch + 1) * 512], in0=ps[:],
                                scalar1=bo_sb[:, ot:ot + 1])
                    nc.sync.dma_start(out=out_ext[b, ot * 128:(ot + 1) * 128, :], in_=osb[:])

            def emit_all():
                # software pipelining: batch 0 starts attention before its v
                # path is built (pair-0 AVs deferred); batch 1's front is
                # emitted in two chunks mid-attention(0); outproj(0) in two
                # chunks mid-attention(1)
                fin_box = [None, None]
                emit_consts_early()
                xb0, qk2_0 = emit_front_xqk(0)
                d0 = emit_head(0, qk2_0, fin_box, defer_av=True)
                d1 = emit_head(1, qk2_0, fin_box, defer_av=True)
                vp0, v1t_0 = emit_front_v(0, xb0)
                emit_consts_late()
                fin_box[1] = v1t_0
                a65_00, rz_00 = d0(v1t_0)
                a65_01, rz_01 = d1(v1t_0)
                if cfg.get("dbg"):
                    nc.sync.dma_start(out=dbga_ext[0, :, :], in_=a65_00[:])
                    nc.sync.dma_start(out=dbga_ext[1, :, :], in_=a65_01[:])
                box = {}

                def cb_front1():
                    box["xqk"] = emit_front_xqk(1, defer=True)
                    box["v"] = emit_front_v(1, box["xqk"][0], defer=True)

                enh0, pend = emit_attn(
                    0, qk2_0, vp0, v1t_0, fin_box,
                    cbs={1: cb_front1},
                    first_pair=([a65_00, a65_01], [rz_00, rz_01]))
                qk2_1 = box["xqk"][1]
                vp1, v1t_1 = box["v"]
                enh1, _ = emit_attn(
                    1, qk2_1, vp1, v1t_1, fin_box,
                    cbs={1: lambda: emit_outproj(0, enh0, ots=(0, 1), defer=True),
                         2: lambda: emit_outproj(0, enh0, ots=(2, 3), defer=True)},
                    pending=pend, final=True)
                flush_defq()
                emit_outproj(1, enh1, wide=True)

            if loop_k is None:
                emit_all()
            else:
                with tc.For_i(0, loop_k, 1):
                    emit_all()

    nc.finalize()
    return nc


def _host_prep(w_qkv, g_qkv, b_qkv, w_pos, g_pos, b_pos, w_out, g_out, b_out):
    bf16 = ml_dtypes.bfloat16
    perm_q = np.empty(256, np.int64)
    perm_k = np.empty(256, np.int64)
    for t in range(2):
        for p in range(128):
            h = 4 * t + p // 32
            d = p % 32
            perm_q[t * 128 + p] = h * 128 + d
            perm_k[t * 128 + p] = h * 128 + 32 + d
    perm_qk = np.concatenate([perm_q, perm_k])
    perm_v = np.array([h * 128 + 64 + d for h in range(8) for d in range(64)])

    wg = (w_qkv * g_qkv[:, None]).astype(np.float32)
    wqkT = np.ascontiguousarray(wg[perm_qk].T).astype(bf16)
    wvT = np.ascontiguousarray(wg[perm_v].T).astype(bf16)
    woT = np.ascontiguousarray((w_out * g_out[:, None]).T).astype(bf16)

    def pack_bias(v):
        return np.ascontiguousarray(v.reshape(4, 128).T).astype(np.float32)

    wpos = (w_pos[:, 0] * g_pos[:, None, None]).astype(np.float32)  # [512, 3, 3]
    wdiag = np.zeros((4, 9, 128, 128), np.float32)
    idx = np.arange(128)
    for t in range(4):
        for ti, (dy, dx) in enumerate((dy, dx) for dy in (-1, 0, 1) for dx in (-1, 0, 1)):
            wdiag[t, ti, idx, idx] = wpos[t * 128:(t + 1) * 128, dy + 1, dx + 1]
    wdiag = wdiag.astype(bf16)
    # negated bf16-rounded weights for corrections: [128, 4*9]
    wneg = np.zeros((128, 36), np.float32)
    for t in range(4):
        for ti in range(9):
            dy, dx = ti // 3 - 1, ti % 3 - 1
            wneg[:, t * 9 + ti] = -wpos[t * 128:(t + 1) * 128, dy + 1, dx + 1] \
                .astype(bf16).astype(np.float32)

    return dict(
        wqkT=wqkT, wvT=wvT, woT=woT,
        bqk=pack_bias(b_qkv[perm_qk]), bv=pack_bias(b_qkv[perm_v]),
        bo=pack_bias(b_out), bpos=pack_bias(b_pos),
        wdiag=wdiag, wneg=wneg, wposc=-wneg,
        ident=np.eye(128, dtype=bf16),
        ones64=np.concatenate(
            [np.zeros((64, 128), np.float32), np.ones((1, 128), np.float32)]
        ).astype(bf16),
    )


def kernel(x, w_qkv, g_qkv, b_qkv, w_pos, g_pos, b_pos, w_out, g_out, b_out,
           _trace=False):
    from concourse.bass_utils import run_bass_kernel_spmd

    x = np.asarray(x, np.float32)
    B, Cin, H, W = x.shape
    assert (B, Cin, H, W) == (16, 512, 32, 32)

    if "nc" not in _cache:
        _cache["nc"] = _build_nc()
    nc = _cache["nc"]

    prep = _host_prep(np.asarray(w_qkv, np.float32), np.asarray(g_qkv, np.float32),
                      np.asarray(b_qkv, np.float32), np.asarray(w_pos, np.float32),
                      np.asarray(g_pos, np.float32), np.asarray(b_pos, np.float32),
                      np.asarray(w_out, np.float32), np.asarray(g_out, np.float32),
                      np.asarray(b_out, np.float32))

    xs = x.reshape(N_CORES, B_PER_CORE, 512, 1024).astype(ml_dtypes.bfloat16)
    in_maps = [dict(prep, x=np.ascontiguousarray(xs[i])) for i in range(N_CORES)]
    _cache["last_in_maps"] = in_maps
    res = run_bass_kernel_spmd(nc, in_maps, list(range(N_CORES)))
    _cache["last_result"] = res
    out = np.stack([res.results[i]["out"] for i in range(N_CORES)])
    return out.reshape(16, 512, 32, 32).astype(np.float32)


# revision 105
# speedup vs baseline: 1.0823x; 1.0566x over previous
"""Trainium2 Bass kernel for nn_Attention_81484119540519.

8-head attention block over 32x32 spatial (1024 tokens), C=512, B=16:
  qkv = BN(1x1conv(x)); S = q^T k * scale; P = softmax(S); A = v P^T
  pos = BN(depthwise3x3(v)); out = BN(1x1conv(A + pos))

Sharding: pure data-parallel over batch. B=16 -> 2 batches per core on 8
NeuronCores; no collectives. Host prepares permuted/folded weights, each
core computes its 2 batches, host concatenates.

Per-core dataflow (fp32 PSUM accumulation everywhere):
  - qk projection (bf16 matmuls) emits q/k in fp8e4, then SBUF->SBUF
    DMAs repack each head block to [16, 2, n] (channel pairs per
    partition, 32-aligned bases) for the PE's fp8 DoubleRow mode.
  - scores q^T k run as DoubleRow matmuls (K=16x2, 2x row rate,
    measured 2.07x on HW); exp on ScalarE straight out of PSUM (scale
    folded into the activation), writing fp8e4 E into [128, 2048]
    key-m-PAIR tiles.
  - v projection in natural channel order bf16 (the depthwise conv and
    pos branch dominate output magnitude, so v stays high precision);
    PE-transposed into fp8 v1T pair tiles [128, 2x640] (head slots
    padded 65->80 for the dual-fp8 Ldweights 16B-alignment rule) with a
    ones column per head: AV runs as DoubleRow matmuls contracting two
    key tiles per instruction, yielding A' = [A; Z] with Z the softmax
    denominator.  fp8 on the E/v1t/qk path costs ~0.4% rel err (the
    attended branch is a convex average, small next to pos).
  - 1/Z via reciprocal_approx_fast on a [2, HW] tile at base partition
    0 (the DVE ISA op corrupts at base 64 on HW), bounced through DRAM
    and broadcast across partitions with a stride-0 DMA read, then one
    tensor_mul per head merges A/Z; conv PSUM + BN bias fold in via
    scalar_tensor_tensor.
  - depthwise 3x3 conv: all 9 taps as diagonal-weight matmuls on
    TensorE over a y-padded image (PE has slack; DVE is the real-HW
    secondary bottleneck); only the x-edge wraparound corrections stay
    on DVE.
  - emission scheduling: the Act engine paces everything (128 exps),
    and the in-order PE queue executes in emission order, so all
    non-score work (pair tails, next batch's front, batch-0 outproj)
    is queued as ~1us chunks on a deque drained one per score slot --
    filling the PE/DVE idle windows under the exp stream instead of
    piling up at batch edges.  Per-head AV(3)+copies thread into the
    next head's emission; batch-0 pair 0 runs scores before the v path
    exists (AVs deferred); the last batch-0 tail is emitted inside
    batch-1's attention.
"""

import numpy as np
import ml_dtypes

NUM_HEADS = 8
KD = 32
HD = 64
C = 512
HW = 1024
SCALE = KD ** -0.5
B_PER_CORE = 2
N_CORES = 8

_cache = {}
CFG = dict(zb_dma=True, pool_dy0=False, az_pool=False, slots=True, z_swdge=True)


def _build_nc(loop_k=None, cfg=None):
    cfg = dict(CFG, **(cfg or {}))
    import concourse.bass as bass
    import concourse.tile as tile
    from concourse import bacc, mybir

    f32 = mybir.dt.float32
    bf16 = mybir.dt.bfloat16
    fp8 = mybir.dt.float8e4
    AF = mybir.ActivationFunctionType
    OP = mybir.AluOpType
    DR = mybir.MatmulPerfMode.DoubleRow

    nc = bacc.Bacc("TRN2", target_bir_lowering=False, debug=False)

    # ---- DRAM parameters (per-core shard + shared prepped weights) ----
    x_ext = nc.declare_dram_parameter("x", [B_PER_CORE, C, HW], bf16, isOutput=False)
    wqk_ext = nc.declare_dram_parameter("wqkT", [C, 512], bf16, isOutput=False)
    wv_ext = nc.declare_dram_parameter("wvT", [C, 512], bf16, isOutput=False)
    wo_ext = nc.declare_dram_parameter("woT", [C, 512], bf16, isOutput=False)
    # biases packed [128, 4] (column t = c-tile t)
    bqk_ext = nc.declare_dram_parameter("bqk", [128, 4], f32, isOutput=False)
    bv_ext = nc.declare_dram_parameter("bv", [128, 4], f32, isOutput=False)
    bo_ext = nc.declare_dram_parameter("bo", [128, 4], f32, isOutput=False)
    bpos_ext = nc.declare_dram_parameter("bpos", [128, 4], f32, isOutput=False)
    # diag conv weights [4 ctiles, 9 taps, 128, 128] bf16
    wdiag_ext = nc.declare_dram_parameter("wdiag", [4, 9, 128, 128], bf16, isOutput=False)
    # negated bf16-rounded conv weights for edge corrections [128, 4*9] f32
    wneg_ext = nc.declare_dram_parameter("wneg", [128, 36], f32, isOutput=False)
    wposc_ext = nc.declare_dram_parameter("wposc", [128, 36], f32, isOutput=False)
    ident_ext = nc.declare_dram_parameter("ident", [128, 128], bf16, isOutput=False)
    ones64_ext = nc.declare_dram_parameter("ones64", [65, 128], bf16, isOutput=False)
    out_ext = nc.declare_dram_parameter("out", [B_PER_CORE, C, HW], bf16, isOutput=True)
    zdram = nc.dram_tensor("zscratch", [16, HW], f32)
    if cfg.get("dbg"):
        dbga_ext = nc.declare_dram_parameter("dbga", [8, 65, HW], f32, isOutput=True)
        dbgrz_ext = nc.declare_dram_parameter("dbgrz", [8, HW], bf16, isOutput=True)
        dbge_ext = nc.declare_dram_parameter("dbge", [4, 128, HW], bf16, isOutput=True)

    NB = B_PER_CORE
    NM = 8           # key m tiles of 128
    NMP = 4          # key m-tile PAIRS
    NCHUNK = 2       # n chunks of 512
    VP = 1120        # padded v row length (zeros at [0,33) and [1057,1120))

    from contextlib import ExitStack

    with tile.TileContext(nc) as tc, ExitStack() as ctx:
        consts = ctx.enter_context(tc.tile_pool(name="consts", bufs=1))
        xbp = ctx.enter_context(tc.tile_pool(name="xb", bufs=8))
        qk8p = ctx.enter_context(tc.tile_pool(name="qk8", bufs=8))
        qk2p = ctx.enter_context(tc.tile_pool(name="qk2", bufs=8))
        vpp = ctx.enter_context(tc.tile_pool(name="vp", bufs=8))
        v1tp = ctx.enter_context(tc.tile_pool(name="v1t", bufs=8))
        ep = ctx.enter_context(tc.tile_pool(name="E", bufs=10))
        a65p = ctx.enter_context(tc.tile_pool(name="a65", bufs=5))
        rcpp = ctx.enter_context(tc.tile_pool(name="rcp", bufs=3))
        enhp = ctx.enter_context(tc.tile_pool(name="enh", bufs=8))
        outp = ctx.enter_context(tc.tile_pool(name="osb", bufs=4))

        s_psum = ctx.enter_context(tc.tile_pool(name="spsum", bufs=2, space="PSUM"))
        acc_psum = ctx.enter_context(tc.tile_pool(name="accpsum", bufs=2, space="PSUM"))
        misc_psum = ctx.enter_context(tc.tile_pool(name="miscpsum", bufs=2, space="PSUM"))

        if True:
            # ---------------- constants (DMA'd in order of first use) ----------
            wqk_sb = consts.tile([128, 4, 512], bf16)
            wv_sb = consts.tile([128, 4, 512], bf16)
            wo_sb = consts.tile([128, 4, 512], bf16)
            bqk_sb = consts.tile([128, 4], f32)
            bv_sb = consts.tile([128, 4], f32)
            bo_sb = consts.tile([128, 4], f32)
            bpos_sb = consts.tile([128, 4], f32)
            wdiag_sb = consts.tile([128, 4, 9, 128], bf16)
            wneg_sb = consts.tile([128, 36], f32)
            wposc_sb = consts.tile([128, 36], f32)
            ident_sb = consts.tile([128, 128], bf16)
            ones64_sb = consts.tile([65, 128], bf16)

            def emit_consts_early():
                nc.sync.dma_start(out=bqk_sb[:], in_=bqk_ext[:])
                for t in range(4):
                    nc.sync.dma_start(out=wqk_sb[:, t, :], in_=wqk_ext[t * 128:(t + 1) * 128, :])

            def emit_consts_v():
                nc.sync.dma_start(out=bv_sb[:], in_=bv_ext[:])
                for t in range(4):
                    nc.sync.dma_start(out=wv_sb[:, t, :], in_=wv_ext[t * 128:(t + 1) * 128, :])
                nc.sync.dma_start(out=ident_sb[:], in_=ident_ext[:])
                nc.sync.dma_start(out=ones64_sb[:], in_=ones64_ext[:])

            def emit_consts_late():
                nc.sync.dma_start(out=bpos_sb[:], in_=bpos_ext[:])
                nc.sync.dma_start(out=wneg_sb[:], in_=wneg_ext[:])
                nc.sync.dma_start(out=wposc_sb[:], in_=wposc_ext[:])
                for t in range(4):
                    nc.sync.dma_start(out=wdiag_sb[:, t, :, :], in_=wdiag_ext[t, :, :, :].rearrange("k p f -> p k f"))
                nc.sync.dma_start(out=bo_sb[:], in_=bo_ext[:])
                for t in range(4):
                    nc.sync.dma_start(out=wo_sb[:, t, :], in_=wo_ext[t * 128:(t + 1) * 128, :])

            def emit_front_xqk(b, ots=(0, 2, 1, 3), xb_t=None, defer=False):
                """x DMA, qk projection + fp8 repack for the given o-tiles.
                defer=True queues the work as defq chunks (next batch's
                front drains through the current batch's score slots)."""
                if xb_t is None:
                    xb_t = []
                    for kt in range(4):
                        xb = xbp.tile([128, HW], bf16)
                        # SWDGE: keeps x off the HWDGE queues that carry
                        # weights, so batch 0's projection isn't stuck
                        # behind const loads
                        nc.gpsimd.dma_start(out=xb[:], in_=x_ext[b, kt * 128:(kt + 1) * 128, :])
                        xb_t.append(xb)

                qk2_t = [None] * 4
                qk8_t = [None] * 4

                def repack(ot, blk):
                    # repack for DoubleRow: head block blk (32 channels) ->
                    # [16, 2, HW] (channel d=2p+i on partition p, slot i).
                    # AP base partitions only allow {0, 32, 64}, so blocks
                    # go in two [64, .] tiles at bases 0 and 32.
                    eng = nc.gpsimd if cfg.get("repack_swdge") else nc.sync
                    eng.dma_start(
                        out=qk2_t[ot][blk // 2][32 * (blk % 2): 32 * (blk % 2) + 16, :],
                        in_=qk8_t[ot][32 * blk: 32 * blk + 32, :])

                def proj_ch(ot, ch):
                    ps = misc_psum.tile([128, 512], f32, tag="mm")
                    for kt in range(4):
                        nc.tensor.matmul(
                            ps[:], wqk_sb[:, kt, ot * 128:(ot + 1) * 128],
                            xb_t[kt][:, ch * 512:(ch + 1) * 512],
                            start=(kt == 0), stop=(kt == 3))
                    nc.vector.tensor_scalar_add(
                        out=qk8_t[ot][:, ch * 512:(ch + 1) * 512], in0=ps[:],
                        scalar1=bqk_sb[:, ot:ot + 1])

                for ot in ots:  # head 0 needs tiles 0 (q) and 2 (k) first
                    qk8_t[ot] = qk8p.tile([128, HW], fp8, name="qk8")
                    qk2_t[ot] = [qk2p.tile([64, 2 * HW], fp8, name="qk2")
                                 for _ in range(2)]

                def unit(f):
                    qchunks([f]) if defer else f()

                for ot in ots:
                    unit(lambda ot=ot: proj_ch(ot, 0))
                    # blocks 0,1 (first head pair of each tile) repacked
                    # right after the tile; blocks 2,3 can trail
                    unit(lambda ot=ot: (proj_ch(ot, 1), repack(ot, 0),
                                        repack(ot, 1)))
                for ot in ots:
                    unit(lambda ot=ot: (repack(ot, 2), repack(ot, 3)))
                return xb_t, qk2_t

            def emit_front_v(b, xb_t, defer=False):
                """v projection (padded spatial layout) + v1T pair tiles."""
                if b == 0:
                    emit_consts_v()
                vp_t = [vpp.tile([128, VP], bf16, name="vp") for _ in range(4)]
                # v1T pair tiles: [128, 2x520-ish] fp8, m=2mp+i at free
                # 640*i, head h at h*80 (padded 65->80 so DoubleRow
                # Ldweights sees 16B-aligned offsets), ones column at +64.
                v1t_mp = [v1tp.tile([128, 1280], fp8, name="v1t")
                          for _ in range(NMP)]

                def unit(f):
                    qchunks([f]) if defer else f()

                def vproj_ch(ot, ch):
                    if ch == 0:
                        nc.gpsimd.memset(vp_t[ot][:, 0:33], 0.0)
                        nc.gpsimd.memset(vp_t[ot][:, 1057:1120], 0.0)
                    ps = misc_psum.tile([128, 512], f32, tag="mm")
                    for kt in range(4):
                        nc.tensor.matmul(
                            ps[:], wv_sb[:, kt, ot * 128:(ot + 1) * 128],
                            xb_t[kt][:, ch * 512:(ch + 1) * 512],
                            start=(kt == 0), stop=(kt == 3))
                    nc.vector.tensor_scalar_add(
                        out=vp_t[ot][:, 33 + ch * 512: 33 + (ch + 1) * 512], in0=ps[:],
                        scalar1=bv_sb[:, ot:ot + 1])

                def v1t_half(mp, par):
                    v1t = v1t_mp[mp]
                    if par == 0:
                        nc.gpsimd.memset(
                            v1t.rearrange("p (s c) -> p s c", s=16)[:, :, 64:65], 1.0)
                    m = 2 * mp + par
                    for ct in range(4):
                        tp = misc_psum.tile([128, 128], bf16, tag="mm")
                        nc.tensor.transpose(
                            tp[:], vp_t[ct][:, 33 + m * 128: 33 + (m + 1) * 128],
                            ident_sb[:])
                        nc.vector.tensor_copy(
                            out=v1t[:, 640 * par + 160 * ct: 640 * par + 160 * ct + 160]
                                .rearrange("p (s c) -> p s c", s=2)[:, :, 0:64],
                            in_=tp.rearrange("p (s c) -> p s c", s=2)[:, :, :])

                for ot in range(4):
                    for ch in range(NCHUNK):
                        unit(lambda ot=ot, ch=ch: vproj_ch(ot, ch))
                for mp in range(NMP):
                    for par in range(2):
                        unit(lambda mp=mp, par=par: v1t_half(mp, par))
                return vp_t, v1t_mp

            import collections
            defq = collections.deque()

            def qchunks(chunks):
                """Queue chunks for slot-draining, or emit inline when the
                slot mechanism is disabled (ablation)."""
                if cfg["slots"]:
                    defq.extend(chunks)
                else:
                    for c in chunks:
                        c()

            def fill_slot():
                """Emit one deferred chunk into the engines' idle window
                (the PE is Act-paced during scores, so deferred tail /
                outproj / next-front work queued here lands in gaps instead
                of piling up at the end of the batch).  Drains two chunks
                when backlogged."""
                if not cfg["slots"]:
                    return
                if defq:
                    defq.popleft()()
                if len(defq) > 8:
                    defq.popleft()()

            def flush_defq():
                while defq:
                    defq.popleft()()

            def emit_pair_tail(b, ct, pair_a65, pair_rz, vp_t):
                """normalize pair ct (PE base-64 ones broadcast of 1/Z from
                the a65 tile's own partition-64 row — no partition-moving
                DMA), then this c-tile's depthwise conv and merge.  Work is
                queued as small chunks on defq, drained one per score slot."""
                enh = enhp.tile([128, HW], bf16)

                def c_z(ch):
                    for hh in range(2):
                        zps = misc_psum.tile([128, 512], f32, tag="mm")
                        nc.tensor.matmul(
                            zps[:], ones64_sb[64:65, :],
                            pair_rz[hh][64:65, ch * 512:(ch + 1) * 512],
                            start=True, stop=True)
                        rcp = rcpp.tile([128, 512], f32, name="rcp")
                        nc.vector.reciprocal_approx_fast(out=rcp[:], in_=zps[:])
                        nc.vector.tensor_mul(
                            out=enh[hh * 64:(hh + 1) * 64, ch * 512:(ch + 1) * 512],
                            in0=pair_a65[hh][0:64, ch * 512:(ch + 1) * 512],
                            in1=rcp[0:64, :])

                def c_zdram_w():
                    # proven old scheme: Z rows DMA'd to partitions 0/1,
                    # recip [2, HW], bounce via DRAM
                    eng = nc.gpsimd if cfg.get("zw_swdge") else nc.sync
                    zpair = rcpp.tile([2, HW], f32, name="zpair", tag="zpair")
                    for hh in range(2):
                        eng.dma_start(out=zpair[hh:hh + 1, :],
                                      in_=pair_a65[hh][64:65, :])
                    rzpair = rcpp.tile([2, HW], f32, name="rzpair", tag="rzpair")
                    nc.vector.reciprocal_approx_fast(out=rzpair[:], in_=zpair[:])
                    p0b = b * 8 + 2 * ct
                    eng.dma_start(out=zdram[p0b:p0b + 2, :], in_=rzpair[:])

                def c_zdram(hh):
                    p0b = b * 8 + 2 * ct
                    zrow = zdram[p0b + hh:p0b + hh + 1, :]
                    bcast = bass.AP(tensor=zrow.tensor, offset=zrow.offset,
                                    ap=[[0, 64]] + list(zrow.ap[1:]))
                    zb = rcpp.tile([64, HW], f32, name="zb")
                    # big broadcast reads optionally on the SWDGE queue to
                    # relieve the sync HWDGE queue (outputs/repacks/consts)
                    eng = nc.gpsimd if cfg.get("z_swdge") else nc.sync
                    eng.dma_start(out=zb[:], in_=bcast)
                    nc.vector.tensor_mul(
                        out=enh[hh * 64:(hh + 1) * 64, :],
                        in0=pair_a65[hh][0:64, :], in1=zb[:])

                dve_dy0 = cfg["pool_dy0"] and ct != 3
                pe_taps = [(ti, dy, dx) for ti, (dy, dx) in enumerate(
                    (dy, dx) for dy in (-1, 0, 1) for dx in (-1, 0, 1))
                    if not (dve_dy0 and dy == 0)]

                def c_conv(ch):
                    ps = misc_psum.tile([128, 512], f32, tag="mm")
                    for j, (ti, dy, dx) in enumerate(pe_taps):
                        off = 33 + 32 * dy + dx + ch * 512
                        nc.tensor.matmul(
                            ps[:], wdiag_sb[:, ct, ti, :],
                            vp_t[ct][:, off:off + 512],
                            start=(j == 0), stop=(j == len(pe_taps) - 1))
                    nc.vector.scalar_tensor_tensor(
                        out=enh[:, ch * 512:(ch + 1) * 512],
                        in0=ps[:], scalar=bpos_sb[:, ct:ct + 1],
                        in1=enh[:, ch * 512:(ch + 1) * 512],
                        op0=OP.add, op1=OP.add)

                def c_dy0():
                    # dy=0 conv row on DVE: enh += w * v (taps 3,4,5)
                    for dx in (-1, 0, 1):
                        ti = 3 + dx + 1
                        nc.vector.scalar_tensor_tensor(
                            out=enh[:], in0=vp_t[ct][:, 33 + dx: 33 + dx + HW],
                            scalar=wposc_sb[:, ct * 9 + ti: ct * 9 + ti + 1],
                            in1=enh[:], op0=OP.mult, op1=OP.add)

                def c_corr():
                    # x-wraparound corrections (dx = +/-1 taps)
                    for dy in (-1, 0, 1):
                        ys = [y for y in range(32) if 0 <= y + dy + 1 < 32]
                        y0, cnt = ys[0], len(ys)
                        oc = enh[:, y0 * 32: (y0 + cnt) * 32] \
                            .rearrange("p (a o) -> p a o", o=32)[:, :, 31:32]
                        sc = vp_t[ct][:, 33 + (y0 + dy + 1) * 32: 33 + (y0 + dy + 1 + cnt) * 32] \
                            .rearrange("p (a o) -> p a o", o=32)[:, :, 0:1]
                        nc.vector.scalar_tensor_tensor(
                            out=oc, in0=sc,
                            scalar=wneg_sb[:, ct * 9 + (dy + 1) * 3 + 2: ct * 9 + (dy + 1) * 3 + 3],
                            in1=oc, op0=OP.mult, op1=OP.add)
                        ys = [y for y in range(32) if 0 <= y + dy - 1 < 32]
                        y0, cnt = ys[0], len(ys)
                        oc = enh[:, y0 * 32: (y0 + cnt) * 32] \
                            .rearrange("p (a o) -> p a o", o=32)[:, :, 0:1]
                        sc = vp_t[ct][:, 33 + (y0 + dy - 1) * 32: 33 + (y0 + dy - 1 + cnt) * 32] \
                            .rearrange("p (a o) -> p a o", o=32)[:, :, 31:32]
                        nc.vector.scalar_tensor_tensor(
                            out=oc, in0=sc,
                            scalar=wneg_sb[:, ct * 9 + (dy + 1) * 3: ct * 9 + (dy + 1) * 3 + 1],
                            in1=oc, op0=OP.mult, op1=OP.add)

                if cfg["zb_dma"]:
                    chunks = [c_zdram_w, lambda: c_zdram(0), lambda: c_zdram(1),
                              lambda: c_conv(0), lambda: c_conv(1)]
                else:
                    chunks = [lambda: c_z(0), lambda: c_z(1),
                              lambda: c_conv(0), lambda: c_conv(1)]
                if dve_dy0:
                    chunks.append(c_dy0)
                chunks.append(c_corr)
                if cfg.get("dbg") and b == 0:
                    chunks.append(lambda: nc.sync.dma_start(
                        out=dbge_ext[ct, :, :], in_=enh[:]))
                qchunks(chunks)
                return enh

            def emit_head(h, qk2_t, fin_box, defer_av=False):
                """One head: DoubleRow scores per key-m tile, exp into fp8
                m-pair E tiles, DoubleRow AV contracting a pair per matmul.
                AV(mp) is emitted after scores(2mp+2) so the PE never waits
                on the exp of the pair it is about to consume; AV(3) plus
                the PSUM->SBUF copies move into a `finisher` that the NEXT
                head emits after its first exp, so the head-boundary
                Act->PE->Act latency hides under live work.  `fin_box`
                carries that pending finisher.  With defer_av the caller
                gets a closure emitting all AVs later (batch-0 startup:
                scores can run before the v path exists)."""
                t = h // 4
                pq = 32 * (h % 2)
                q2 = qk2_t[t][(h % 4) // 2].rearrange("p (s n) -> p s n", s=2)[pq:pq + 16, :, :]
                k2 = qk2_t[2 + t][(h % 4) // 2].rearrange("p (s n) -> p s n", s=2)[pq:pq + 16, :, :]
                a65c = []
                e2_mp = [None] * NMP

                def av(mp, v1t_mp):
                    if mp == 0:
                        a65c.extend(acc_psum.tile([65, 512], f32, name="a65c",
                                                  tag="a65c") for _ in range(NCHUNK))
                    w2 = v1t_mp[mp].rearrange("p (s c) -> p s c", s=2)[:, :, h * 80:h * 80 + 65]
                    e2v = e2_mp[mp].rearrange("p (s n) -> p s n", s=2)
                    for ch in range(NCHUNK):
                        nc.tensor.matmul(
                            a65c[ch][:], w2, e2v[:, :, ch * 512:(ch + 1) * 512],
                            start=(mp == 0), stop=(mp == NMP - 1), perf_mode=DR)

                def finish(v1t_mp):
                    av(NMP - 1, v1t_mp)
                    a65_sb = a65p.tile([65, HW], f32, name="a65_sb")
                    for ch in range(NCHUNK):
                        nc.vector.tensor_copy(
                            out=a65_sb[:, ch * 512:(ch + 1) * 512], in_=a65c[ch][:])
                    if cfg["zb_dma"]:
                        # Z reaches the tail via the a65 row-64 DMA; no
                        # bf16 Z-row copy needed
                        return a65_sb, None
                    # Z row to bf16 (plain DVE copy handles base partition
                    # 64; the reciprocal ISA op does NOT on hardware, so
                    # 1/Z is taken after the base-0 broadcast instead)
                    zb = a65p.tile([65, HW], bf16, name="zb16", tag="zb16")
                    nc.vector.tensor_copy(out=zb[64:65, :], in_=a65_sb[64:65, :])
                    return a65_sb, zb

                for m in range(NM):
                    mp, par = m // 2, m % 2
                    st = s_psum.tile([128, HW], f32, name="st")
                    for ch in range(NCHUNK):
                        nc.tensor.matmul(
                            st[:, ch * 512:(ch + 1) * 512],
                            k2[:, :, m * 128:(m + 1) * 128],
                            q2[:, :, ch * 512:(ch + 1) * 512],
                            start=True, stop=True, perf_mode=DR)
                    if par == 0:
                        e2_mp[mp] = ep.tile([128, 2 * HW], fp8, name="e2")
                    nc.scalar.activation(
                        out=e2_mp[mp][:, par * HW:(par + 1) * HW], in_=st[:],
                        func=AF.Exp, scale=float(SCALE))
                    if m == 1 and fin_box[0] is not None:
                        f, fin_box[0] = fin_box[0], None
                        f()
                    if not defer_av and m in (3, 5, 7):
                        av((m - 3) // 2, fin_box[1])
                    if m >= 2:
                        fill_slot()

                if defer_av:
                    def deferred(v1t_mp):
                        for mp in range(NMP - 1):
                            av(mp, v1t_mp)
                        return finish(v1t_mp)
                    return deferred
                return finish

            def emit_attn(b, qk2_t, vp_t, v1t_mp, fin_box, cbs,
                          first_pair=None, pending=None, final=False):
                """Pairs of heads; each pair's tail chunks are queued half a
                pair later and drained by subsequent score slots.  cbs maps
                hp -> callback emitted after the pair.  first_pair: prebuilt
                (pair_a65, zpair) for batch-0's deferred pair 0.  pending
                carries the last tail's (dest, args) across batches."""
                enh_t = []
                fin_box[1] = v1t_mp
                start_hp = 0

                def mk_fin(finish, pair_a65, pair_rz, hh, v1t, h):
                    def f():
                        a65_sb, rz = finish(v1t)
                        pair_a65[hh] = a65_sb
                        pair_rz[hh] = rz
                        if cfg.get("dbg") and b == 0:
                            nc.sync.dma_start(out=dbga_ext[h, :, :], in_=a65_sb[:])
                            if rz is not None:
                                nc.sync.dma_start(out=dbgrz_ext[h:h + 1, :],
                                                  in_=rz[64:65, :])
                    return f  # noqa: the rz here is the bf16 Z row tile

                if first_pair is not None:
                    pending = (enh_t, (b, 0, *first_pair, vp_t))
                    start_hp = 1
                for hp in range(start_hp, 4):
                    pair_a65 = [None, None]
                    pair_rz = [None, None]
                    for hh in range(2):
                        finish = emit_head(2 * hp + hh, qk2_t, fin_box)
                        fin_box[0] = mk_fin(finish, pair_a65, pair_rz, hh, v1t_mp,
                                            2 * hp + hh)
                        if hh == 0 and pending is not None:
                            # queue the previous pair's tail chunks half a
                            # pair late: far enough that their z-chain deps
                            # are ready when drain slots reach them, early
                            # enough that slots exist to drain them
                            dest, args = pending
                            dest.append(emit_pair_tail(*args))
                            pending = None
                    pending = (enh_t, (b, hp, pair_a65, pair_rz, vp_t))
                    if hp in cbs:
                        cbs[hp]()
                if final:
                    if fin_box[0] is not None:
                        f, fin_box[0] = fin_box[0], None
                        f()
                    dest, args = pending
                    dest.append(emit_pair_tail(*args))
                    pending = None
                return enh_t, pending

            def emit_outproj(b, enh_t, wide=False, ots=(0, 1, 2, 3), defer=False):
                # wide=True: attention is over, borrow the idle s_pool banks
                # for 2 full o-tiles in flight.  defer=True queues one chunk
                # per o-tile on defq instead of emitting inline.
                if defer:
                    for ot in ots:
                        qchunks([lambda ot=ot: emit_outproj(b, enh_t, ots=(ot,))])
                    return
                for ot in ots:
                    osb = outp.tile([128, HW], bf16)
                    if wide:
                        pw = s_psum.tile([128, HW], f32, tag="st", name="st")
                        for ch in range(NCHUNK):
                            for kt in range(4):
                                nc.tensor.matmul(
                                    pw[:, ch * 512:(ch + 1) * 512],
                                    wo_sb[:, kt, ot * 128:(ot + 1) * 128],
                                    enh_t[kt][:, ch * 512:(ch + 1) * 512],
                                    start=(kt == 0), stop=(kt == 3))
                        nc.vector.tensor_scalar_add(
                            out=osb[:], in0=pw[:], scalar1=bo_sb[:, ot:ot + 1])
                    else:
                        for ch in range(NCHUNK):
                            ps = misc_psum.tile([128, 512], f32, tag="mm")
                            for kt in range(4):
                                nc.tensor.matmul(
                                    ps[:], wo_sb[:, kt, ot * 128:(ot + 1) * 128],
                                    enh_t[kt][:, ch * 512:(ch + 1) * 512],
                                    start=(kt == 0), stop=(kt == 3))
                            nc.vector.tensor_scalar_add(
                                out=osb[:, ch * 512:(ch + 1) * 512], in0=ps[:],
                                scalar1=bo_sb[:, ot:ot + 1])
                    nc.sync.dma_start(out=out_ext[b, ot * 128:(ot + 1) * 128, :], in_=osb[:])

            def emit_all():
                # software pipelining: batch 0 starts attention before its v
                # path is built (pair-0 AVs deferred); batch 1's front is
                # emitted in two chunks mid-attention(0); outproj(0) in two
                # chunks mid-attention(1)
                fin_box = [None, None]
                emit_consts_early()
                xb0, qk2_0 = emit_front_xqk(0)
                d0 = emit_head(0, qk2_0, fin_box, defer_av=True)
                d1 = emit_head(1, qk2_0, fin_box, defer_av=True)
                vp0, v1t_0 = emit_front_v(0, xb0)
                emit_consts_late()
                fin_box[1] = v1t_0
                a65_00, rz_00 = d0(v1t_0)
                a65_01, rz_01 = d1(v1t_0)
                if cfg.get("dbg"):
                    nc.sync.dma_start(out=dbga_ext[0, :, :], in_=a65_00[:])
                    nc.sync.dma_start(out=dbga_ext[1, :, :], in_=a65_01[:])
                box = {}

                def cb_front1():
                    box["xqk"] = emit_front_xqk(1, defer=True)
                    box["v"] = emit_front_v(1, box["xqk"][0], defer=True)

                enh0, pend = emit_attn(
                    0, qk2_0, vp0, v1t_0, fin_box,
                    cbs={1: cb_front1},
                    first_pair=([a65_00, a65_01], [rz_00, rz_01]))
                qk2_1 = box["xqk"][1]
                vp1, v1t_1 = box["v"]
                enh1, _ = emit_attn(
                    1, qk2_1, vp1, v1t_1, fin_box,
                    cbs={1: lambda: emit_outproj(0, enh0, ots=(0, 1), defer=True),
                         2: lambda: emit_outproj(0, enh0, ots=(2, 3), defer=True)},
                    pending=pend, final=True)
                flush_defq()
                emit_outproj(1, enh1, wide=True)

            if loop_k is None:
                emit_all()
            else:
                with tc.For_i(0, loop_k, 1):
                    emit_all()

    nc.finalize()
    return nc


def _host_prep(w_qkv, g_qkv, b_qkv, w_pos, g_pos, b_pos, w_out, g_out, b_out):
    bf16 = ml_dtypes.bfloat16
    perm_q = np.empty(256, np.int64)
    perm_k = np.empty(256, np.int64)
    for t in range(2):
        for p in range(128):
            h = 4 * t + p // 32
            d = p % 32
            perm_q[t * 128 + p] = h * 128 + d
            perm_k[t * 128 + p] = h * 128 + 32 + d
    perm_qk = np.concatenate([perm_q, perm_k])
    perm_v = np.array([h * 128 + 64 + d for h in range(8) for d in range(64)])

    wg = (w_qkv * g_qkv[:, None]).astype(np.float32)
    wqkT = np.ascontiguousarray(wg[perm_qk].T).astype(bf16)
    wvT = np.ascontiguousarray(wg[perm_v].T).astype(bf16)
    woT = np.ascontiguousarray((w_out * g_out[:, None]).T).astype(bf16)

    def pack_bias(v):
        return np.ascontiguousarray(v.reshape(4, 128).T).astype(np.float32)

    wpos = (w_pos[:, 0] * g_pos[:, None, None]).astype(np.float32)  # [512, 3, 3]
    wdiag = np.zeros((4, 9, 128, 128), np.float32)
    idx = np.arange(128)
    for t in range(4):
        for ti, (dy, dx) in enumerate((dy, dx) for dy in (-1, 0, 1) for dx in (-1, 0, 1)):
            wdiag[t, ti, idx, idx] = wpos[t * 128:(t + 1) * 128, dy + 1, dx + 1]
    wdiag = wdiag.astype(bf16)
    # negated bf16-rounded weights for corrections: [128, 4*9]
    wneg = np.zeros((128, 36), np.float32)
    for t in range(4):
        for ti in range(9):
            dy, dx = ti // 3 - 1, ti % 3 - 1
            wneg[:, t * 9 + ti] = -wpos[t * 128:(t + 1) * 128, dy + 1, dx + 1] \
                .astype(bf16).astype(np.float32)

    return dict(
        wqkT=wqkT, wvT=wvT, woT=woT,
        bqk=pack_bias(b_qkv[perm_qk]), bv=pack_bias(b_qkv[perm_v]),
        bo=pack_bias(b_out), bpos=pack_bias(b_pos),
        wdiag=wdiag, wneg=wneg, wposc=-wneg,
        ident=np.eye(128, dtype=bf16),
        ones64=np.concatenate(
            [np.zeros((64, 128), np.float32), np.ones((1, 128), np.float32)]
        ).astype(bf16),
    )


def kernel(x, w_qkv, g_qkv, b_qkv, w_pos, g_pos, b_pos, w_out, g_out, b_out,
           _trace=False):
    from concourse.bass_utils import run_bass_kernel_spmd

    x = np.asarray(x, np.float32)
    B, Cin, H, W = x.shape
    assert (B, Cin, H, W) == (16, 512, 32, 32)

    if "nc" not in _cache:
        _cache["nc"] = _build_nc()
    nc = _cache["nc"]

    prep = _host_prep(np.asarray(w_qkv, np.float32), np.asarray(g_qkv, np.float32),
                      np.asarray(b_qkv, np.float32), np.asarray(w_pos, np.float32),
                      np.asarray(g_pos, np.float32), np.asarray(b_pos, np.float32),
                      np.asarray(w_out, np.float32), np.asarray(g_out, np.float32),
                      np.asarray(b_out, np.float32))

    xs = x.reshape(N_CORES, B_PER_CORE, 512, 1024).astype(ml_dtypes.bfloat16)
    in_maps = [dict(prep, x=np.ascontiguousarray(xs[i])) for i in range(N_CORES)]
    _cache["last_in_maps"] = in_maps
    res = run_bass_kernel_spmd(nc, in_maps, list(range(N_CORES)))
    _cache["last_result"] = res
    out = np.stack([res.results[i]["out"] for i in range(N_CORES)])
    return out.reshape(16, 512, 32, 32).astype(np.float32)


# revision 108
# speedup vs baseline: 1.1341x; 1.0479x over previous
"""Trainium2 Bass kernel for nn_Attention_81484119540519.

8-head attention block over 32x32 spatial (1024 tokens), C=512, B=16:
  qkv = BN(1x1conv(x)); S = q^T k * scale; P = softmax(S); A = v P^T
  pos = BN(depthwise3x3(v)); out = BN(1x1conv(A + pos))

Sharding: pure data-parallel over batch. B=16 -> 2 batches per core on 8
NeuronCores; no collectives. Host prepares permuted/folded weights, each
core computes its 2 batches, host concatenates.

Per-core dataflow (fp32 PSUM accumulation everywhere):
  - qk projection (bf16 matmuls) emits q/k in fp8e4, then SBUF->SBUF
    DMAs repack each head block to [16, 2, n] (channel pairs per
    partition, 32-aligned bases) for the PE's fp8 DoubleRow mode.
  - scores q^T k run as DoubleRow matmuls (K=16x2, 2x row rate,
    measured 2.07x on HW); exp on ScalarE straight out of PSUM (scale
    folded into the activation), writing fp8e4 E into [128, 2048]
    key-m-PAIR tiles.
  - v projection in natural channel order bf16 (the depthwise conv and
    pos branch dominate output magnitude, so v stays high precision);
    PE-transposed into fp8 v1T pair tiles [128, 2x640] (head slots
    padded 65->80 for the dual-fp8 Ldweights 16B-alignment rule) with a
    ones column per head: AV runs as DoubleRow matmuls contracting two
    key tiles per instruction, yielding A' = [A; Z] with Z the softmax
    denominator.  fp8 on the E/v1t/qk path costs ~0.4% rel err (the
    attended branch is a convex average, small next to pos).
  - 1/Z via reciprocal_approx_fast on a [2, HW] tile at base partition
    0 (the DVE ISA op corrupts at base 64 on HW), bounced through DRAM
    and broadcast across partitions with a stride-0 DMA read, then one
    tensor_mul per head merges A/Z; conv PSUM + BN bias fold in via
    scalar_tensor_tensor.
  - depthwise 3x3 conv: all 9 taps as diagonal-weight matmuls on
    TensorE over a y-padded image (PE has slack; DVE is the real-HW
    secondary bottleneck); only the x-edge wraparound corrections stay
    on DVE.
  - emission scheduling: the Act engine paces everything (128 exps),
    and the in-order PE queue executes in emission order, so all
    non-score work (pair tails, next batch's front, batch-0 outproj)
    is queued as ~1us chunks on a deque drained one per score slot --
    filling the PE/DVE idle windows under the exp stream instead of
    piling up at batch edges.  Per-head AV(3)+copies thread into the
    next head's emission; batch-0 pair 0 runs scores before the v path
    exists (AVs deferred); the last batch-0 tail is emitted inside
    batch-1's attention.
"""

import numpy as np
import ml_dtypes

NUM_HEADS = 8
KD = 32
HD = 64
C = 512
HW = 1024
SCALE = KD ** -0.5
B_PER_CORE = 2
N_CORES = 8

_cache = {}
CFG = dict(zb_dma=True, pool_dy0=False, az_pool=False, slots=True, z_swdge=True,
           drain_hi=4, ep_bufs=14)


def _build_nc(loop_k=None, cfg=None):
    cfg = dict(CFG, **(cfg or {}))
    import concourse.bass as bass
    import concourse.tile as tile
    from concourse import bacc, mybir

    f32 = mybir.dt.float32
    bf16 = mybir.dt.bfloat16
    fp8 = mybir.dt.float8e4
    AF = mybir.ActivationFunctionType
    OP = mybir.AluOpType
    DR = mybir.MatmulPerfMode.DoubleRow

    nc = bacc.Bacc("TRN2", target_bir_lowering=False, debug=False)

    # ---- DRAM parameters (per-core shard + shared prepped weights) ----
    x_ext = nc.declare_dram_parameter("x", [B_PER_CORE, C, HW], bf16, isOutput=False)
    wqk_ext = nc.declare_dram_parameter("wqkT", [C, 512], bf16, isOutput=False)
    wv_ext = nc.declare_dram_parameter("wvT", [C, 512], bf16, isOutput=False)
    wo_ext = nc.declare_dram_parameter("woT", [C, 512], bf16, isOutput=False)
    # biases packed [128, 4] (column t = c-tile t)
    bqk_ext = nc.declare_dram_parameter("bqk", [128, 4], f32, isOutput=False)
    bv_ext = nc.declare_dram_parameter("bv", [128, 4], f32, isOutput=False)
    bo_ext = nc.declare_dram_parameter("bo", [128, 4], f32, isOutput=False)
    bpos_ext = nc.declare_dram_parameter("bpos", [128, 4], f32, isOutput=False)
    # diag conv weights [4 ctiles, 9 taps, 128, 128] bf16
    wdiag_ext = nc.declare_dram_parameter("wdiag", [4, 9, 128, 128], bf16, isOutput=False)
    # negated bf16-rounded conv weights for edge corrections [128, 4*9] f32
    wneg_ext = nc.declare_dram_parameter("wneg", [128, 36], f32, isOutput=False)
    wposc_ext = nc.declare_dram_parameter("wposc", [128, 36], f32, isOutput=False)
    ident_ext = nc.declare_dram_parameter("ident", [128, 128], bf16, isOutput=False)
    ones64_ext = nc.declare_dram_parameter("ones64", [65, 128], bf16, isOutput=False)
    out_ext = nc.declare_dram_parameter("out", [B_PER_CORE, C, HW], bf16, isOutput=True)
    zdram = nc.dram_tensor("zscratch", [16, HW], f32)
    if cfg.get("dbg"):
        dbga_ext = nc.declare_dram_parameter("dbga", [8, 65, HW], f32, isOutput=True)
        dbgrz_ext = nc.declare_dram_parameter("dbgrz", [8, HW], bf16, isOutput=True)
        dbge_ext = nc.declare_dram_parameter("dbge", [4, 128, HW], bf16, isOutput=True)

    NB = B_PER_CORE
    NM = 8           # key m tiles of 128
    NMP = 4          # key m-tile PAIRS
    NCHUNK = 2       # n chunks of 512
    VP = 1120        # padded v row length (zeros at [0,33) and [1057,1120))

    from contextlib import ExitStack

    with tile.TileContext(nc) as tc, ExitStack() as ctx:
        consts = ctx.enter_context(tc.tile_pool(name="consts", bufs=1))
        xbp = ctx.enter_context(tc.tile_pool(name="xb", bufs=8))
        qk8p = ctx.enter_context(tc.tile_pool(name="qk8", bufs=8))
        qk2p = ctx.enter_context(tc.tile_pool(name="qk2", bufs=8))
        vpp = ctx.enter_context(tc.tile_pool(name="vp", bufs=8))
        v1tp = ctx.enter_context(tc.tile_pool(name="v1t", bufs=8))
        ep = ctx.enter_context(tc.tile_pool(name="E", bufs=cfg.get("ep_bufs", 10)))
        a65p = ctx.enter_context(tc.tile_pool(name="a65", bufs=5))
        rcpp = ctx.enter_context(tc.tile_pool(name="rcp", bufs=3))
        enhp = ctx.enter_context(tc.tile_pool(name="enh", bufs=8))
        outp = ctx.enter_context(tc.tile_pool(name="osb", bufs=4))

        s_psum = ctx.enter_context(tc.tile_pool(name="spsum", bufs=2, space="PSUM"))
        acc_psum = ctx.enter_context(tc.tile_pool(name="accpsum", bufs=2, space="PSUM"))
        misc_psum = ctx.enter_context(tc.tile_pool(name="miscpsum", bufs=2, space="PSUM"))

        if True:
            # ---------------- constants (DMA'd in order of first use) ----------
            wqk_sb = consts.tile([128, 4, 512], bf16)
            wv_sb = consts.tile([128, 4, 512], bf16)
            wo_sb = consts.tile([128, 4, 512], bf16)
            bqk_sb = consts.tile([128, 4], f32)
            bv_sb = consts.tile([128, 4], f32)
            bo_sb = consts.tile([128, 4], f32)
            bpos_sb = consts.tile([128, 4], f32)
            wdiag_sb = consts.tile([128, 4, 9, 128], bf16)
            wneg_sb = consts.tile([128, 36], f32)
            wposc_sb = consts.tile([128, 36], f32)
            ident_sb = consts.tile([128, 128], bf16)
            ones64_sb = consts.tile([65, 128], bf16)

            def emit_consts_early():
                nc.sync.dma_start(out=bqk_sb[:], in_=bqk_ext[:])
                for t in range(4):
                    nc.sync.dma_start(out=wqk_sb[:, t, :], in_=wqk_ext[t * 128:(t + 1) * 128, :])

            def emit_consts_v():
                nc.sync.dma_start(out=bv_sb[:], in_=bv_ext[:])
                for t in range(4):
                    nc.sync.dma_start(out=wv_sb[:, t, :], in_=wv_ext[t * 128:(t + 1) * 128, :])
                nc.sync.dma_start(out=ident_sb[:], in_=ident_ext[:])
                nc.sync.dma_start(out=ones64_sb[:], in_=ones64_ext[:])

            def emit_consts_late():
                nc.sync.dma_start(out=bpos_sb[:], in_=bpos_ext[:])
                nc.sync.dma_start(out=wneg_sb[:], in_=wneg_ext[:])
                nc.sync.dma_start(out=wposc_sb[:], in_=wposc_ext[:])
                for t in range(4):
                    nc.sync.dma_start(out=wdiag_sb[:, t, :, :], in_=wdiag_ext[t, :, :, :].rearrange("k p f -> p k f"))
                nc.sync.dma_start(out=bo_sb[:], in_=bo_ext[:])
                for t in range(4):
                    nc.sync.dma_start(out=wo_sb[:, t, :], in_=wo_ext[t * 128:(t + 1) * 128, :])

            def emit_front_xqk(b, ots=(0, 2, 1, 3), xb_t=None, defer=False):
                """x DMA, qk projection + fp8 repack for the given o-tiles.
                defer=True queues the work as defq chunks (next batch's
                front drains through the current batch's score slots)."""
                if xb_t is None:
                    xb_t = []
                    for kt in range(4):
                        xb = xbp.tile([128, HW], bf16)
                        # SWDGE: keeps x off the HWDGE queues that carry
                        # weights, so batch 0's projection isn't stuck
                        # behind const loads
                        nc.gpsimd.dma_start(out=xb[:], in_=x_ext[b, kt * 128:(kt + 1) * 128, :])
                        xb_t.append(xb)

                qk2_t = [None] * 4
                qk8_t = [None] * 4

                def repack(ot, blk):
                    # repack for DoubleRow: head block blk (32 channels) ->
                    # [16, 2, HW] (channel d=2p+i on partition p, slot i).
                    # AP base partitions only allow {0, 32, 64}, so blocks
                    # go in two [64, .] tiles at bases 0 and 32.
                    eng = nc.gpsimd if cfg.get("repack_swdge") else nc.sync
                    eng.dma_start(
                        out=qk2_t[ot][blk // 2][32 * (blk % 2): 32 * (blk % 2) + 16, :],
                        in_=qk8_t[ot][32 * blk: 32 * blk + 32, :])

                def proj_ch(ot, ch):
                    ps = misc_psum.tile([128, 512], f32, tag="mm")
                    for kt in range(4):
                        nc.tensor.matmul(
                            ps[:], wqk_sb[:, kt, ot * 128:(ot + 1) * 128],
                            xb_t[kt][:, ch * 512:(ch + 1) * 512],
                            start=(kt == 0), stop=(kt == 3))
                    nc.vector.tensor_scalar_add(
                        out=qk8_t[ot][:, ch * 512:(ch + 1) * 512], in0=ps[:],
                        scalar1=bqk_sb[:, ot:ot + 1])

                for ot in ots:  # head 0 needs tiles 0 (q) and 2 (k) first
                    qk8_t[ot] = qk8p.tile([128, HW], fp8, name="qk8")
                    qk2_t[ot] = [qk2p.tile([64, 2 * HW], fp8, name="qk2")
                                 for _ in range(2)]

                def unit(f):
                    qchunks([f]) if defer else f()

                for ot in ots:
                    unit(lambda ot=ot: proj_ch(ot, 0))
                    # blocks 0,1 (first head pair of each tile) repacked
                    # right after the tile; blocks 2,3 can trail
                    unit(lambda ot=ot: (proj_ch(ot, 1), repack(ot, 0),
                                        repack(ot, 1)))
                for ot in ots:
                    unit(lambda ot=ot: (repack(ot, 2), repack(ot, 3)))
                return xb_t, qk2_t

            def emit_front_v(b, xb_t, defer=False):
                """v projection (padded spatial layout) + v1T pair tiles."""
                if b == 0:
                    emit_consts_v()
                vp_t = [vpp.tile([128, VP], bf16, name="vp") for _ in range(4)]
                # v1T pair tiles: [128, 2x520-ish] fp8, m=2mp+i at free
                # 640*i, head h at h*80 (padded 65->80 so DoubleRow
                # Ldweights sees 16B-aligned offsets), ones column at +64.
                v1t_mp = [v1tp.tile([128, 1280], fp8, name="v1t")
                          for _ in range(NMP)]

                def unit(f):
                    qchunks([f]) if defer else f()

                def vproj_ch(ot, ch):
                    if ch == 0:
                        nc.gpsimd.memset(vp_t[ot][:, 0:33], 0.0)
                        nc.gpsimd.memset(vp_t[ot][:, 1057:1120], 0.0)
                    ps = misc_psum.tile([128, 512], f32, tag="mm")
                    for kt in range(4):
                        nc.tensor.matmul(
                            ps[:], wv_sb[:, kt, ot * 128:(ot + 1) * 128],
                            xb_t[kt][:, ch * 512:(ch + 1) * 512],
                            start=(kt == 0), stop=(kt == 3))
                    nc.vector.tensor_scalar_add(
                        out=vp_t[ot][:, 33 + ch * 512: 33 + (ch + 1) * 512], in0=ps[:],
                        scalar1=bv_sb[:, ot:ot + 1])

                def v1t_half(mp, par):
                    v1t = v1t_mp[mp]
                    if par == 0:
                        nc.gpsimd.memset(
                            v1t.rearrange("p (s c) -> p s c", s=16)[:, :, 64:65], 1.0)
                    m = 2 * mp + par
                    for ct in range(4):
                        tp = misc_psum.tile([128, 128], bf16, tag="mm")
                        nc.tensor.transpose(
                            tp[:], vp_t[ct][:, 33 + m * 128: 33 + (m + 1) * 128],
                            ident_sb[:])
                        nc.vector.tensor_copy(
                            out=v1t[:, 640 * par + 160 * ct: 640 * par + 160 * ct + 160]
                                .rearrange("p (s c) -> p s c", s=2)[:, :, 0:64],
                            in_=tp.rearrange("p (s c) -> p s c", s=2)[:, :, :])

                for ot in range(4):
                    for ch in range(NCHUNK):
                        unit(lambda ot=ot, ch=ch: vproj_ch(ot, ch))
                for mp in range(NMP):
                    for par in range(2):
                        unit(lambda mp=mp, par=par: v1t_half(mp, par))
                return vp_t, v1t_mp

            import collections
            defq = collections.deque()

            def qchunks(chunks):
                """Queue chunks for slot-draining, or emit inline when the
                slot mechanism is disabled (ablation)."""
                if cfg["slots"]:
                    defq.extend(chunks)
                else:
                    for c in chunks:
                        c()

            def fill_slot():
                """Emit one deferred chunk into the engines' idle window
                (the PE is Act-paced during scores, so deferred tail /
                outproj / next-front work queued here lands in gaps instead
                of piling up at the end of the batch).  Drains two chunks
                when backlogged."""
                if not cfg["slots"]:
                    return
                if defq:
                    defq.popleft()()
                if len(defq) > cfg.get("drain_hi", 8):
                    defq.popleft()()

            def flush_defq():
                while defq:
                    defq.popleft()()

            def emit_pair_tail(b, ct, pair_a65, pair_rz, vp_t):
                """normalize pair ct (PE base-64 ones broadcast of 1/Z from
                the a65 tile's own partition-64 row — no partition-moving
                DMA), then this c-tile's depthwise conv and merge.  Work is
                queued as small chunks on defq, drained one per score slot."""
                enh = enhp.tile([128, HW], bf16)

                def c_z(ch):
                    for hh in range(2):
                        zps = misc_psum.tile([128, 512], f32, tag="mm")
                        nc.tensor.matmul(
                            zps[:], ones64_sb[64:65, :],
                            pair_rz[hh][64:65, ch * 512:(ch + 1) * 512],
                            start=True, stop=True)
                        rcp = rcpp.tile([128, 512], f32, name="rcp")
                        nc.vector.reciprocal_approx_fast(out=rcp[:], in_=zps[:])
                        nc.vector.tensor_mul(
                            out=enh[hh * 64:(hh + 1) * 64, ch * 512:(ch + 1) * 512],
                            in0=pair_a65[hh][0:64, ch * 512:(ch + 1) * 512],
                            in1=rcp[0:64, :])

                def c_zdram_w():
                    # proven old scheme: Z rows DMA'd to partitions 0/1,
                    # recip [2, HW], bounce via DRAM
                    eng = nc.gpsimd if cfg.get("zw_swdge") else nc.sync
                    zpair = rcpp.tile([2, HW], f32, name="zpair", tag="zpair")
                    for hh in range(2):
                        eng.dma_start(out=zpair[hh:hh + 1, :],
                                      in_=pair_a65[hh][64:65, :])
                    rzpair = rcpp.tile([2, HW], f32, name="rzpair", tag="rzpair")
                    nc.vector.reciprocal_approx_fast(out=rzpair[:], in_=zpair[:])
                    p0b = b * 8 + 2 * ct
                    eng.dma_start(out=zdram[p0b:p0b + 2, :], in_=rzpair[:])

                def c_zdram(hh):
                    p0b = b * 8 + 2 * ct
                    zrow = zdram[p0b + hh:p0b + hh + 1, :]
                    bcast = bass.AP(tensor=zrow.tensor, offset=zrow.offset,
                                    ap=[[0, 64]] + list(zrow.ap[1:]))
                    zb = rcpp.tile([64, HW], f32, name="zb")
                    # big broadcast reads optionally on the SWDGE queue to
                    # relieve the sync HWDGE queue (outputs/repacks/consts)
                    eng = nc.gpsimd if cfg.get("z_swdge") else nc.sync
                    eng.dma_start(out=zb[:], in_=bcast)
                    nc.vector.tensor_mul(
                        out=enh[hh * 64:(hh + 1) * 64, :],
                        in0=pair_a65[hh][0:64, :], in1=zb[:])

                dve_dy0 = cfg["pool_dy0"] and ct != 3
                pe_taps = [(ti, dy, dx) for ti, (dy, dx) in enumerate(
                    (dy, dx) for dy in (-1, 0, 1) for dx in (-1, 0, 1))
                    if not (dve_dy0 and dy == 0)]

                def c_conv(ch):
                    ps = misc_psum.tile([128, 512], f32, tag="mm")
                    for j, (ti, dy, dx) in enumerate(pe_taps):
                        off = 33 + 32 * dy + dx + ch * 512
                        nc.tensor.matmul(
                            ps[:], wdiag_sb[:, ct, ti, :],
                            vp_t[ct][:, off:off + 512],
                            start=(j == 0), stop=(j == len(pe_taps) - 1))
                    nc.vector.scalar_tensor_tensor(
                        out=enh[:, ch * 512:(ch + 1) * 512],
                        in0=ps[:], scalar=bpos_sb[:, ct:ct + 1],
                        in1=enh[:, ch * 512:(ch + 1) * 512],
                        op0=OP.add, op1=OP.add)

                def c_dy0():
                    # dy=0 conv row on DVE: enh += w * v (taps 3,4,5)
                    for dx in (-1, 0, 1):
                        ti = 3 + dx + 1
                        nc.vector.scalar_tensor_tensor(
                            out=enh[:], in0=vp_t[ct][:, 33 + dx: 33 + dx + HW],
                            scalar=wposc_sb[:, ct * 9 + ti: ct * 9 + ti + 1],
                            in1=enh[:], op0=OP.mult, op1=OP.add)

                def c_corr():
                    # x-wraparound corrections (dx = +/-1 taps)
                    for dy in (-1, 0, 1):
                        ys = [y for y in range(32) if 0 <= y + dy + 1 < 32]
                        y0, cnt = ys[0], len(ys)
                        oc = enh[:, y0 * 32: (y0 + cnt) * 32] \
                            .rearrange("p (a o) -> p a o", o=32)[:, :, 31:32]
                        sc = vp_t[ct][:, 33 + (y0 + dy + 1) * 32: 33 + (y0 + dy + 1 + cnt) * 32] \
                            .rearrange("p (a o) -> p a o", o=32)[:, :, 0:1]
                        nc.vector.scalar_tensor_tensor(
                            out=oc, in0=sc,
                            scalar=wneg_sb[:, ct * 9 + (dy + 1) * 3 + 2: ct * 9 + (dy + 1) * 3 + 3],
                            in1=oc, op0=OP.mult, op1=OP.add)
                        ys = [y for y in range(32) if 0 <= y + dy - 1 < 32]
                        y0, cnt = ys[0], len(ys)
                        oc = enh[:, y0 * 32: (y0 + cnt) * 32] \
                            .rearrange("p (a o) -> p a o", o=32)[:, :, 0:1]
                        sc = vp_t[ct][:, 33 + (y0 + dy - 1) * 32: 33 + (y0 + dy - 1 + cnt) * 32] \
                            .rearrange("p (a o) -> p a o", o=32)[:, :, 31:32]
                        nc.vector.scalar_tensor_tensor(
                            out=oc, in0=sc,
                            scalar=wneg_sb[:, ct * 9 + (dy + 1) * 3: ct * 9 + (dy + 1) * 3 + 1],
                            in1=oc, op0=OP.mult, op1=OP.add)

                if cfg["zb_dma"]:
                    chunks = [c_zdram_w, lambda: c_zdram(0), lambda: c_zdram(1),
                              lambda: c_conv(0), lambda: c_conv(1)]
                else:
                    chunks = [lambda: c_z(0), lambda: c_z(1),
                              lambda: c_conv(0), lambda: c_conv(1)]
                if dve_dy0:
                    chunks.append(c_dy0)
                chunks.append(c_corr)
                if cfg.get("dbg") and b == 0:
                    chunks.append(lambda: nc.sync.dma_start(
                        out=dbge_ext[ct, :, :], in_=enh[:]))
                qchunks(chunks)
                return enh

            def emit_head(h, qk2_t, fin_box, defer_av=False):
                """One head: DoubleRow scores per key-m tile, exp into fp8
                m-pair E tiles, DoubleRow AV contracting a pair per matmul.
                AV(mp) is emitted after scores(2mp+2) so the PE never waits
                on the exp of the pair it is about to consume; AV(3) plus
                the PSUM->SBUF copies move into a `finisher` that the NEXT
                head emits after its first exp, so the head-boundary
                Act->PE->Act latency hides under live work.  `fin_box`
                carries that pending finisher.  With defer_av the caller
                gets a closure emitting all AVs later (batch-0 startup:
                scores can run before the v path exists)."""
                t = h // 4
                pq = 32 * (h % 2)
                q2 = qk2_t[t][(h % 4) // 2].rearrange("p (s n) -> p s n", s=2)[pq:pq + 16, :, :]
                k2 = qk2_t[2 + t][(h % 4) // 2].rearrange("p (s n) -> p s n", s=2)[pq:pq + 16, :, :]
                a65c = []
                e2_mp = [None] * NMP

                def av(mp, v1t_mp):
                    if mp == 0:
                        a65c.extend(acc_psum.tile([65, 512], f32, name="a65c",
                                                  tag="a65c") for _ in range(NCHUNK))
                    w2 = v1t_mp[mp].rearrange("p (s c) -> p s c", s=2)[:, :, h * 80:h * 80 + 65]
                    e2v = e2_mp[mp].rearrange("p (s n) -> p s n", s=2)
                    for ch in range(NCHUNK):
                        nc.tensor.matmul(
                            a65c[ch][:], w2, e2v[:, :, ch * 512:(ch + 1) * 512],
                            start=(mp == 0), stop=(mp == NMP - 1), perf_mode=DR)

                def finish(v1t_mp):
                    av(NMP - 1, v1t_mp)
                    a65_sb = a65p.tile([65, HW], f32, name="a65_sb")
                    for ch in range(NCHUNK):
                        nc.vector.tensor_copy(
                            out=a65_sb[:, ch * 512:(ch + 1) * 512], in_=a65c[ch][:])
                    if cfg["zb_dma"]:
                        # Z reaches the tail via the a65 row-64 DMA; no
                        # bf16 Z-row copy needed
                        return a65_sb, None
                    # Z row to bf16 (plain DVE copy handles base partition
                    # 64; the reciprocal ISA op does NOT on hardware, so
                    # 1/Z is taken after the base-0 broadcast instead)
                    zb = a65p.tile([65, HW], bf16, name="zb16", tag="zb16")
                    nc.vector.tensor_copy(out=zb[64:65, :], in_=a65_sb[64:65, :])
                    return a65_sb, zb

                for m in range(NM):
                    mp, par = m // 2, m % 2
                    st = s_psum.tile([128, HW], f32, name="st")
                    for ch in range(NCHUNK):
                        nc.tensor.matmul(
                            st[:, ch * 512:(ch + 1) * 512],
                            k2[:, :, m * 128:(m + 1) * 128],
                            q2[:, :, ch * 512:(ch + 1) * 512],
                            start=True, stop=True, perf_mode=DR)
                    if par == 0:
                        e2_mp[mp] = ep.tile([128, 2 * HW], fp8, name="e2")
                    nc.scalar.activation(
                        out=e2_mp[mp][:, par * HW:(par + 1) * HW], in_=st[:],
                        func=AF.Exp, scale=float(SCALE))
                    if m == 1 and fin_box[0] is not None:
                        f, fin_box[0] = fin_box[0], None
                        f()
                    if not defer_av and m in (3, 5, 7):
                        av((m - 3) // 2, fin_box[1])
                    if m >= 2:
                        fill_slot()

                if defer_av:
                    def deferred(v1t_mp):
                        for mp in range(NMP - 1):
                            av(mp, v1t_mp)
                        return finish(v1t_mp)
                    return deferred
                return finish

            def emit_attn(b, qk2_t, vp_t, v1t_mp, fin_box, cbs,
                          first_pair=None, pending=None, final=False):
                """Pairs of heads; each pair's tail chunks are queued half a
                pair later and drained by subsequent score slots.  cbs maps
                hp -> callback emitted after the pair.  first_pair: prebuilt
                (pair_a65, zpair) for batch-0's deferred pair 0.  pending
                carries the last tail's (dest, args) across batches."""
                enh_t = []
                fin_box[1] = v1t_mp
                start_hp = 0

                def mk_fin(finish, pair_a65, pair_rz, hh, v1t, h):
                    def f():
                        a65_sb, rz = finish(v1t)
                        pair_a65[hh] = a65_sb
                        pair_rz[hh] = rz
                        if cfg.get("dbg") and b == 0:
                            nc.sync.dma_start(out=dbga_ext[h, :, :], in_=a65_sb[:])
                            if rz is not None:
                                nc.sync.dma_start(out=dbgrz_ext[h:h + 1, :],
                                                  in_=rz[64:65, :])
                    return f  # noqa: the rz here is the bf16 Z row tile

                if first_pair is not None:
                    pending = (enh_t, (b, 0, *first_pair, vp_t))
                    start_hp = 1
                for hp in range(start_hp, 4):
                    pair_a65 = [None, None]
                    pair_rz = [None, None]
                    for hh in range(2):
                        finish = emit_head(2 * hp + hh, qk2_t, fin_box)
                        fin_box[0] = mk_fin(finish, pair_a65, pair_rz, hh, v1t_mp,
                                            2 * hp + hh)
                        if hh == 0 and pending is not None:
                            # queue the previous pair's tail chunks half a
                            # pair late: far enough that their z-chain deps
                            # are ready when drain slots reach them, early
                            # enough that slots exist to drain them
                            dest, args = pending
                            dest.append(emit_pair_tail(*args))
                            pending = None
                    pending = (enh_t, (b, hp, pair_a65, pair_rz, vp_t))
                    if hp in cbs:
                        cbs[hp]()
                if final:
                    if fin_box[0] is not None:
                        f, fin_box[0] = fin_box[0], None
                        f()
                    dest, args = pending
                    dest.append(emit_pair_tail(*args))
                    pending = None
                return enh_t, pending

            def emit_outproj(b, enh_t, wide=False, ots=(0, 1, 2, 3), defer=False):
                # wide=True: attention is over, borrow the idle s_pool banks
                # for 2 full o-tiles in flight.  defer=True queues one chunk
                # per o-tile on defq instead of emitting inline.
                if defer:
                    for ot in ots:
                        qchunks([lambda ot=ot: emit_outproj(b, enh_t, ots=(ot,))])
                    return
                for ot in ots:
                    osb = outp.tile([128, HW], bf16)
                    if wide:
                        pw = s_psum.tile([128, HW], f32, tag="st", name="st")
                        for ch in range(NCHUNK):
                            for kt in range(4):
                                nc.tensor.matmul(
                                    pw[:, ch * 512:(ch + 1) * 512],
                                    wo_sb[:, kt, ot * 128:(ot + 1) * 128],
                                    enh_t[kt][:, ch * 512:(ch + 1) * 512],
                                    start=(kt == 0), stop=(kt == 3))
                        nc.vector.tensor_scalar_add(
                            out=osb[:], in0=pw[:], scalar1=bo_sb[:, ot:ot + 1])
                    else:
                        for ch in range(NCHUNK):
                            ps = misc_psum.tile([128, 512], f32, tag="mm")
                            for kt in range(4):
                                nc.tensor.matmul(
                                    ps[:], wo_sb[:, kt, ot * 128:(ot + 1) * 128],
                                    enh_t[kt][:, ch * 512:(ch + 1) * 512],
                                    start=(kt == 0), stop=(kt == 3))
                            nc.vector.tensor_scalar_add(
                                out=osb[:, ch * 512:(ch + 1) * 512], in0=ps[:],
                                scalar1=bo_sb[:, ot:ot + 1])
                    nc.sync.dma_start(out=out_ext[b, ot * 128:(ot + 1) * 128, :], in_=osb[:])

            def emit_all():
                # software pipelining: batch 0 starts attention before its v
                # path is built (pair-0 AVs deferred); batch 1's front is
                # emitted in two chunks mid-attention(0); outproj(0) in two
                # chunks mid-attention(1)
                fin_box = [None, None]
                emit_consts_early()
                xb0, qk2_0 = emit_front_xqk(0)
                d0 = emit_head(0, qk2_0, fin_box, defer_av=True)
                d1 = emit_head(1, qk2_0, fin_box, defer_av=True)
                vp0, v1t_0 = emit_front_v(0, xb0)
                emit_consts_late()
                fin_box[1] = v1t_0
                a65_00, rz_00 = d0(v1t_0)
                a65_01, rz_01 = d1(v1t_0)
                if cfg.get("dbg"):
                    nc.sync.dma_start(out=dbga_ext[0, :, :], in_=a65_00[:])
                    nc.sync.dma_start(out=dbga_ext[1, :, :], in_=a65_01[:])
                box = {}

                def cb_front1():
                    box["xqk"] = emit_front_xqk(1, defer=True)
                    box["v"] = emit_front_v(1, box["xqk"][0], defer=True)

                enh0, pend = emit_attn(
                    0, qk2_0, vp0, v1t_0, fin_box,
                    cbs={1: cb_front1},
                    first_pair=([a65_00, a65_01], [rz_00, rz_01]))
                qk2_1 = box["xqk"][1]
                vp1, v1t_1 = box["v"]
                enh1, _ = emit_attn(
                    1, qk2_1, vp1, v1t_1, fin_box,
                    cbs={1: lambda: emit_outproj(0, enh0, ots=(0, 1), defer=True),
                         2: lambda: emit_outproj(0, enh0, ots=(2, 3), defer=True)},
                    pending=pend, final=True)
                flush_defq()
                emit_outproj(1, enh1, wide=True)

            if loop_k is None:
                emit_all()
            else:
                with tc.For_i(0, loop_k, 1):
                    emit_all()

    nc.finalize()
    return nc


def _host_prep(w_qkv, g_qkv, b_qkv, w_pos, g_pos, b_pos, w_out, g_out, b_out):
    bf16 = ml_dtypes.bfloat16
    perm_q = np.empty(256, np.int64)
    perm_k = np.empty(256, np.int64)
    for t in range(2):
        for p in range(128):
            h = 4 * t + p // 32
            d = p % 32
            perm_q[t * 128 + p] = h * 128 + d
            perm_k[t * 128 + p] = h * 128 + 32 + d
    perm_qk = np.concatenate([perm_q, perm_k])
    perm_v = np.array([h * 128 + 64 + d for h in range(8) for d in range(64)])

    wg = (w_qkv * g_qkv[:, None]).astype(np.float32)
    wqkT = np.ascontiguousarray(wg[perm_qk].T).astype(bf16)
    wvT = np.ascontiguousarray(wg[perm_v].T).astype(bf16)
    woT = np.ascontiguousarray((w_out * g_out[:, None]).T).astype(bf16)

    def pack_bias(v):
        return np.ascontiguousarray(v.reshape(4, 128).T).astype(np.float32)

    wpos = (w_pos[:, 0] * g_pos[:, None, None]).astype(np.float32)  # [512, 3, 3]
    wdiag = np.zeros((4, 9, 128, 128), np.float32)
    idx = np.arange(128)
    for t in range(4):
        for ti, (dy, dx) in enumerate((dy, dx) for dy in (-1, 0, 1) for dx in (-1, 0, 1)):
            wdiag[t, ti, idx, idx] = wpos[t * 128:(t + 1) * 128, dy + 1, dx + 1]
    wdiag = wdiag.astype(bf16)
    # negated bf16-rounded weights for corrections: [128, 4*9]
    wneg = np.zeros((128, 36), np.float32)
    for t in range(4):
        for ti in range(9):
            dy, dx = ti // 3 - 1, ti % 3 - 1
            wneg[:, t * 9 + ti] = -wpos[t * 128:(t + 1) * 128, dy + 1, dx + 1] \
                .astype(bf16).astype(np.float32)

    return dict(
        wqkT=wqkT, wvT=wvT, woT=woT,
        bqk=pack_bias(b_qkv[perm_qk]), bv=pack_bias(b_qkv[perm_v]),
        bo=pack_bias(b_out), bpos=pack_bias(b_pos),
        wdiag=wdiag, wneg=wneg, wposc=-wneg,
        ident=np.eye(128, dtype=bf16),
        ones64=np.concatenate(
            [np.zeros((64, 128), np.float32), np.ones((1, 128), np.float32)]
        ).astype(bf16),
    )


def kernel(x, w_qkv, g_qkv, b_qkv, w_pos, g_pos, b_pos, w_out, g_out, b_out,
           _trace=False):
    from concourse.bass_utils import run_bass_kernel_spmd

    x = np.asarray(x, np.float32)
    B, Cin, H, W = x.shape
    assert (B, Cin, H, W) == (16, 512, 32, 32)

    if "nc" not in _cache:
        _cache["nc"] = _build_nc()
    nc = _cache["nc"]

    prep = _host_prep(np.asarray(w_qkv, np.float32), np.asarray(g_qkv, np.float32),
                      np.asarray(b_qkv, np.float32), np.asarray(w_pos, np.float32),
                      np.asarray(g_pos, np.float32), np.asarray(b_pos, np.float32),
                      np.asarray(w_out, np.float32), np.asarray(g_out, np.float32),
                      np.asarray(b_out, np.float32))

    xs = x.reshape(N_CORES, B_PER_CORE, 512, 1024).astype(ml_dtypes.bfloat16)
    in_maps = [dict(prep, x=np.ascontiguousarray(xs[i])) for i in range(N_CORES)]
    _cache["last_in_maps"] = in_maps
    res = run_bass_kernel_spmd(nc, in_maps, list(range(N_CORES)))
    _cache["last_result"] = res
    out = np.stack([res.results[i]["out"] for i in range(N_CORES)])
    return out.reshape(16, 512, 32, 32).astype(np.float32)
